# revision 51
# baseline (speedup 1.0000x reference)
"""Multi-head attention (B=2, S=2048, D=1024, H=16, dh=64) on 8 Trainium2 cores.

Sharding: head-tensor-parallel x batch. Core c owns batch b=c//4 and heads
4*(c%4)..4*(c%4)+3 (256 of the 1024 ctx dims). Each core computes its heads'
Q/K/V projections, attention, and a partial output projection against its
256 rows of Wo (+ bo/4 so the 4 partials per batch sum to one bo). The host
unshard step sums the 4 partial outputs per batch (the tensor-parallel
all-reduce), done at gather time.

Per-core kernel (fp16 matmul operands, fp32 PSUM accumulation):
  qT/kT [256e, 2048t] = W.T @ x.T computed directly in transposed form so
  scores^T [kt, qt] = (kT slice).T @ (qT slice) needs no on-device transpose.
  Head pairs are row-packed (heads at array rows 0-63 / 64-127) so the two
  K=64 score matmuls of a pair run concurrently via tile_position row groups.
  exp runs on ScalarE with the 1/sqrt(dh) scale folded in. A@V uses a
  stationary operand [V | 1] (ones column injected by the eviction mask-add)
  so the softmax denominator falls out of the same matmul. The denominator
  reciprocal is 1/s = exp(-ln(s)) in two ScalarE ops (same table set as the
  scores exp), woven into the next slice behind queued score exps; the
  1/rowsum row is broadcast across partitions with a K=1 fp16 matmul.

Schedule: DMA order is arranged so the first exp fires ~7us in (wq, wk, x
column-slice 0 first; v/k/q projections stream in JIT as later x column
slices land). Projection/output-projection matmul chains are chopped into
2-3 matmul chunks woven between attention m-steps so the exp cadence never
breaks. Each attn1 slice carries the previous slice's normalization and one
query-slice of output projection; only the last q-slice's projection remains
for the tail.
"""

import numpy as np

import bass_rust
import concourse.bass as bass
import concourse.mybir as mybir
import concourse.tile as tile
from concourse.bass_utils import run_bass_kernel_spmd

B = 2
S = 2048
D = 1024
H = 16
DH = 64
OUT = 1024
NCORES = 8
HPC = H // 4  # heads per core = 4
E = HPC * DH  # 256 ctx dims per core
EV = HPC * (DH + 1)  # 260: v with interleaved ones columns

FP16 = mybir.dt.float16  # fp16: same PE rate as bf16, 10-bit mantissa
FP32 = mybir.dt.float32
F32R = mybir.dt.float32r
I32 = mybir.dt.int32
ALU = mybir.AluOpType

SCALE = 1.0 / float(np.sqrt(DH))
# Reciprocal bit-trick seed: bitcast(~i + RECIP_C) == -(~5%-accurate 1/d)
# for positive d (Schraudolph constant with the sign bit folded in); one
# Newton step squares the error to ~2.6e-3, well under the accuracy gate.
RECIP_C = int(np.int32(np.uint32((0x7EF311C3 + 0x80000000 + 1) & 0xFFFFFFFF)))

KT = D // 128  # 8 k-tiles for projections
MT = S // 128  # 16 key-token tiles
NQ = S // 512  # 4 query slices of 512


def _split_waits(nc, maxw=1):
    """This container's walrus rejects instructions carrying more than one
    semaphore wait ("Too many sync wait commands"); hoist extras onto
    standalone same-engine nops, preserving per-engine program order."""
    for bb in nc.main_func.blocks:
        new_il = []
        for inst in bb.instructions:
            si = inst.sync_info
            if si is not None and si.on_wait and len(si.on_wait) > maxw:
                waits = list(si.on_wait)
                for j, w in enumerate(waits[:-maxw]):
                    nop = mybir.InstNoOp(
                        name=f"{inst.name}-ws{j}", ins=[], outs=[], engine=inst.engine
                    )
                    nop.sync_info = bass_rust.SyncInfo(on_wait=[w], on_update=[])
                    new_il.append(nop)
                inst.sync_info = bass_rust.SyncInfo(
                    on_wait=waits[-maxw:], on_update=list(si.on_update)
                )
            new_il.append(inst)
        bb.instructions = new_il


def build_program():
    nc = bass.Bass()

    xT = nc.declare_dram_parameter("xT", [D, S], FP16, isOutput=False)
    wq = nc.declare_dram_parameter("wq", [D, E], FP16, isOutput=False)
    wk = nc.declare_dram_parameter("wk", [D, E], FP16, isOutput=False)
    wv = nc.declare_dram_parameter("wv", [D, EV], FP16, isOutput=False)
    wo = nc.declare_dram_parameter("wo", [E, OUT], FP16, isOutput=False)
    bqp = nc.declare_dram_parameter("bq", [128, E // 128], FP32, isOutput=False)
    bkp = nc.declare_dram_parameter("bk", [128, E // 128], FP32, isOutput=False)
    mvp = nc.declare_dram_parameter("maskv", [128, EV], FP32, isOutput=False)
    bop = nc.declare_dram_parameter("bo4", [128, OUT // 128], FP32, isOutput=False)
    outT = nc.declare_dram_parameter("outT", [OUT, S], FP16, isOutput=True)

    with tile.TileContext(nc) as tc:
        with (
            tc.tile_pool(name="w", bufs=1) as wpool,
            tc.tile_pool(name="work", bufs=3) as work,
            tc.tile_pool(name="cnp", bufs=1) as cnpool,
            tc.tile_pool(name="ps", bufs=2, space="PSUM") as psp,
            tc.tile_pool(name="ctxps", bufs=2, space="PSUM") as ctxp,
            tc.tile_pool(name="pop", bufs=2, space="PSUM") as pop,
        ):
            # ---- persistent SBUF residents ----
            xts = [wpool.tile([128, S], FP16, tag=f"xt{k}", name=f"xt{k}") for k in range(KT)]
            wqs = [wpool.tile([128, E], FP16, tag=f"wq{k}", name=f"wq{k}") for k in range(KT)]
            wks = [wpool.tile([128, E], FP16, tag=f"wk{k}", name=f"wk{k}") for k in range(KT)]
            wvs = [wpool.tile([128, EV], FP16, tag=f"wv{k}", name=f"wv{k}") for k in range(KT)]
            wos = [wpool.tile([128, OUT], FP16, tag=f"wo{k}", name=f"wo{k}") for k in range(2)]
            bq_s = wpool.tile([128, E // 128], FP32, tag="bq")
            bk_s = wpool.tile([128, E // 128], FP32, tag="bk")
            mv_s = wpool.tile([128, EV], FP32, tag="mv")
            bo_s = wpool.tile([128, OUT // 128], FP32, tag="bo")
            ones_f = wpool.tile([1, 64], FP16, tag="ones_f")
            ones_p = wpool.tile([1, 64], FP16, tag="ones_p")
            qts = [wpool.tile([128, S], FP16, tag=f"qt{m}", name=f"qt{m}") for m in range(2)]
            kts = [wpool.tile([128, S], FP16, tag=f"kt{m}", name=f"kt{m}") for m in range(2)]
            vts = [wpool.tile([128, EV], FP16, tag=f"vt{m}", name=f"vt{m}") for m in range(MT)]
            cns = [cnpool.tile([128, S], FP16, tag=f"cn{m}", name=f"cn{m}") for m in range(2)]

            # DMA issue costs ~650ns each and is serial per engine queue, so
            # the critical first transfers (wq, wk, x column-slice 0) are
            # spread across the three DMA-capable queues (Sync, GpSimd,
            # Scalar) to issue in parallel. Scalar gets only wk so the exp
            # table load / first exp aren't queued behind DMA issues.
            for k in range(KT):
                nc.gpsimd.dma_start(out=wqs[k][:], in_=wq[k * 128 : (k + 1) * 128, :])
            for k in range(KT):
                nc.scalar.dma_start(out=wks[k][:], in_=wk[k * 128 : (k + 1) * 128, :])
            for k in range(KT):
                nc.sync.dma_start(
                    out=xts[k][:, 0:512], in_=xT[k * 128 : (k + 1) * 128, 0:512]
                )
            nc.sync.dma_start(out=bq_s[:], in_=bqp[:])
            nc.sync.dma_start(out=bk_s[:], in_=bkp[:])
            for k in range(KT):
                nc.sync.dma_start(
                    out=xts[k][:, 512:1024], in_=xT[k * 128 : (k + 1) * 128, 512:1024]
                )
            for k in range(KT):
                nc.gpsimd.dma_start(out=wvs[k][:], in_=wv[k * 128 : (k + 1) * 128, :])
            nc.gpsimd.dma_start(out=mv_s[:], in_=mvp[:])
            for k in range(KT):
                nc.gpsimd.dma_start(
                    out=xts[k][:, 1024:1536], in_=xT[k * 128 : (k + 1) * 128, 1024:1536]
                )
            for k in range(KT):
                nc.sync.dma_start(
                    out=xts[k][:, 1536:2048], in_=xT[k * 128 : (k + 1) * 128, 1536:2048]
                )
            for k in range(2):
                nc.gpsimd.dma_start(out=wos[k][:], in_=wo[k * 128 : (k + 1) * 128, :])
            nc.gpsimd.dma_start(out=bo_s[:], in_=bop[:])
            nc.vector.memset(ones_f[:], -1.0)
            nc.vector.memset(ones_p[:], 1.0)

            # Warm the PE clock (HAM un-throttles after ~3.4us sustained)
            # while the first DMAs stream in: no-dependency dummy matmuls.
            wu = wpool.tile([128, 512], FP16, tag="wu")
            nc.vector.memset(wu[:], 0.0)
            wups = psp.tile([128, 1024], FP32, tag="S", name="wups")
            for i in range(16):
                nc.tensor.matmul(
                    wups[:, 0:512], lhsT=wu[:, 0:128], rhs=wu[:], start=True, stop=True
                )

            # ---- projection chain helpers ----
            def qk_burst(hp, which, n):
                """Full 8-matmul projection group (prologue / slice-0 use)."""
                w_s, dst, bias = (
                    (wqs, qts, bq_s) if which == "q" else (wks, kts, bk_s)
                )
                ps = pop.tile([128, 512], FP32, tag="po", name=f"qk_{which}{hp}{n}")
                for k in range(KT):
                    nc.tensor.matmul(
                        ps[:],
                        lhsT=w_s[k][:, hp * 128 : (hp + 1) * 128],
                        rhs=xts[k][:, n * 512 : (n + 1) * 512],
                        start=(k == 0),
                        stop=(k == KT - 1),
                    )
                nc.vector.tensor_add(
                    dst[hp][:, n * 512 : (n + 1) * 512],
                    ps[:],
                    bias[:, hp : hp + 1].to_broadcast((128, 512)),
                )

            def qk_chunks(hp, which, n):
                """The same group as 3 thunks (3+3+2 matmuls) to weave between
                attention m-steps. The accumulator PSUM tile persists across
                chunks (pop ring, bufs=1 serializes chains)."""
                w_s, dst, bias = (
                    (wqs, qts, bq_s) if which == "q" else (wks, kts, bk_s)
                )
                state = {}

                def chunk(ks, first, last):
                    def t():
                        if first:
                            state["ps"] = pop.tile(
                                [128, 512], FP32, tag="po", name=f"qkc_{which}{hp}{n}"
                            )
                        ps = state["ps"]
                        for k in ks:
                            nc.tensor.matmul(
                                ps[:],
                                lhsT=w_s[k][:, hp * 128 : (hp + 1) * 128],
                                rhs=xts[k][:, n * 512 : (n + 1) * 512],
                                start=(k == 0),
                                stop=(k == KT - 1),
                            )
                        if last:
                            nc.vector.tensor_add(
                                dst[hp][:, n * 512 : (n + 1) * 512],
                                ps[:],
                                bias[:, hp : hp + 1].to_broadcast((128, 512)),
                            )
                    return t

                return [
                    chunk(range(0, 3), True, False),
                    chunk(range(3, 6), False, False),
                    chunk(range(6, 8), False, True),
                ]

            def v_group(m):
                """v_ext rows m*128..m*128+127 (token-major); the ones columns
                (and bv) are injected by the mask-add eviction, so no bias
                matmul is needed."""
                ps = pop.tile([128, 512], FP32, tag="po", name=f"psv{m}")
                for k in range(KT):
                    nc.tensor.matmul(
                        ps[:, :EV],
                        lhsT=xts[k][:, m * 128 : (m + 1) * 128],
                        rhs=wvs[k][:],
                        start=(k == 0),
                        stop=(k == KT - 1),
                    )
                nc.vector.tensor_add(vts[m][:], ps[:, :EV], mv_s[:])

            def norm_thunks(ctx_a, ctx_b, hp, nq):
                """The deferred normalization of a finished slice as 5 thunks:
                [p1-evict (DVE), ln (ACT), 1/x exp (ACT), head-a broadcast+
                scale, head-b broadcast+scale]. Spreading them mid-next-slice
                keeps the ACT recip from stalling the score-exp cadence."""
                st = {}

                def p1():
                    st["cs"] = work.tile([65, 1024], FP32, tag="cs", bufs=3, name="cs_ab")
                    nc.vector.tensor_copy(st["cs"][:, 0:512], ctx_a[:])
                    nc.vector.tensor_copy(st["cs"][:, 512:1024], ctx_b[:])
                    # Denominator row replicated to a partition-0 tile (PSUM
                    # source, so the partition remap is legal): the DVE
                    # Newton ops need SBUF operands at matching bases.
                    st["dd"] = work.tile([1, 1024], FP32, tag="dd", bufs=3, name="dd_ab")
                    nc.vector.tensor_copy(st["dd"][0:1, 0:512], ctx_a[64:65, :])
                    nc.vector.tensor_copy(st["dd"][0:1, 512:1024], ctx_b[64:65, :])

                def seed_t():
                    # z = bitcast(~i + C) ~= -1/d, entirely on DVE: the
                    # pacing ACT engine never runs the reciprocal.
                    st["z"] = work.tile([1, 1024], FP32, tag="zz", bufs=3, name="z_ab")
                    zn = work.tile([1, 1024], FP32, tag="zn", bufs=3, name="zn_ab")
                    nc.vector.tensor_scalar(
                        zn[:].bitcast(I32), st["dd"][:].bitcast(I32),
                        0, None, ALU.bitwise_not,
                    )
                    nc.vector.tensor_scalar(
                        st["z"][:].bitcast(I32), zn[:].bitcast(I32),
                        RECIP_C, None, ALU.add,
                    )

                def newt_t():
                    # One Newton step straight to fp16: rr = (d*z + 2)*z,
                    # still -1/d; the -1 ones vector in the broadcast matmul
                    # flips the sign back.
                    tt = work.tile([1, 1024], FP32, tag="tt", bufs=3, name="t_ab")
                    st["rr"] = work.tile([1, 1024], FP16, tag="rr", bufs=3, name="rr_ab")
                    nc.vector.tensor_mul(tt[:], st["dd"][:], st["z"][:])
                    nc.vector.scalar_tensor_tensor(
                        st["rr"][:], tt[:], 2.0, st["z"][:],
                        op0=ALU.add, op1=ALU.mult,
                    )

                def p2(a):
                    def t():
                        bc = pop.tile([128, 512], FP32, tag="po", name=f"bc{hp}{a}{nq}")
                        nc.tensor.matmul(
                            bc[0:64, :],
                            lhsT=ones_f[:],
                            rhs=st["rr"][0:1, 512 * a : 512 * a + 512],
                            start=True,
                            stop=True,
                        )
                        nc.vector.tensor_mul(
                            cns[hp][64 * a : 64 * a + 64, nq * 512 : (nq + 1) * 512],
                            st["cs"][0:64, 512 * a : 512 * a + 512],
                            bc[0:64, :],
                        )
                    return t

                p1.st = st  # the tail reads the cs tile out of the closure
                return [p1, seed_t, newt_t, p2(0), p2(1)]

            def out_proj_group(n, mo, pool=None):
                """One [128,512] tile of the partial out^T for query slice n.
                Output DMA issues alternate between the Sync and GpSimd
                queues (both idle here) so the tail's last issues aren't
                serialized on one queue. `pool` lets the tail alternate PSUM
                rings so back-to-back groups pipeline instead of serializing
                on the single pop buffer."""
                ps = (pool or pop).tile(
                    [128, 512], FP32, tag="S" if pool is not None else "po",
                    name=f"op{n}{mo}",
                )
                for k in range(2):
                    nc.tensor.matmul(
                        ps[:],
                        lhsT=wos[k][:, mo * 128 : (mo + 1) * 128],
                        rhs=cns[k][:, n * 512 : (n + 1) * 512],
                        start=(k == 0),
                        stop=(k == 1),
                    )
                ot = work.tile([128, 512], FP16, tag="ot")
                nc.vector.tensor_scalar_add(ot[:], ps[:], bo_s[:, mo : mo + 1])
                eng = nc.sync if mo % 2 == 0 else nc.gpsimd
                eng.dma_start(
                    out=outT[mo * 128 : (mo + 1) * 128, n * 512 : (n + 1) * 512],
                    in_=ot[:],
                )

            def attn_slice(hp, nq, fillers):
                """One query slice (512) of attention for head pair hp.
                fillers: dict m -> list of thunks emitted just before step m.
                Returns (cs, rr, hp, nq) for the deferred normalization."""
                ctx_a = ctxp.tile([65, 512], FP32, tag="ctx", name="ctx_a")
                ctx_b = ctxp.tile([65, 512], FP32, tag="ctx", name="ctx_b")
                for m in range(MT):
                    sps = psp.tile([128, 1024], FP32, tag="S", name="sps")
                    nc.tensor.matmul(
                        sps[:, 0:512],
                        lhsT=kts[hp][0:64, m * 128 : (m + 1) * 128],
                        rhs=qts[hp][0:64, nq * 512 : (nq + 1) * 512],
                        start=True,
                        stop=True,
                    )
                    nc.tensor.matmul(
                        sps[:, 512:1024],
                        lhsT=kts[hp][64:128, m * 128 : (m + 1) * 128],
                        rhs=qts[hp][64:128, nq * 512 : (nq + 1) * 512],
                        start=True,
                        stop=True,
                    )
                    ee = work.tile([128, 1024], FP16, tag="E", bufs=6)
                    nc.scalar.activation(
                        ee[:], sps[:], mybir.ActivationFunctionType.Exp, scale=SCALE
                    )
                    # Fillers go BETWEEN the scores pair and the A@V pair on
                    # the PE queue: the next step's scores (and so the exp
                    # cadence) never queue behind filler work; only the AVs
                    # slip, absorbed by the ee ring.
                    for f in fillers.get(m, ()):
                        f()
                    ha = 2 * hp
                    nc.tensor.matmul(
                        ctx_a[:],
                        lhsT=vts[m][:, ha * 65 : ha * 65 + 65],
                        rhs=ee[:, 0:512],
                        start=(m == 0),
                        stop=(m == MT - 1),
                    )
                    nc.tensor.matmul(
                        ctx_b[:],
                        lhsT=vts[m][:, (ha + 1) * 65 : (ha + 1) * 65 + 65],
                        rhs=ee[:, 512:1024],
                        start=(m == 0),
                        stop=(m == MT - 1),
                    )
                return norm_thunks(ctx_a, ctx_b, hp, nq)

            def merge(*fds):
                out = {}
                for fd in fds:
                    for k, v in fd.items():
                        out[k] = out.get(k, []) + list(v)
                return out

            def norm_fill(pending, at=(0, 1, 2, 6, 7)):
                """Weave the previous slice's normalization pipeline into
                this slice: DVE evict at 0, ACT ln/exp at 4/5 (behind a few
                queued score exps so the ACT never starves the cadence),
                broadcasts at 6/7."""
                return {s: [t] for s, t in zip(at, pending)}

            def chain_fill(chains, starts):
                """Place each chain's 3 chunks at steps s, s+1, s+2."""
                fd = {}
                for (hp, which, n), s in zip(chains, starts):
                    for i, t in enumerate(qk_chunks(hp, which, n)):
                        fd.setdefault(s + i, []).append(t)
                return fd

            # ---- emission schedule ----
            # Prologue: q0 n=0 and k0 n=0, k-interleaved so both chains ride
            # the same DMA wave (each x k-tile feeds both the q and k mm).
            ps_q = pop.tile([128, 512], FP32, tag="po", name="qk_q00")
            ps_k = pop.tile([128, 512], FP32, tag="po", name="qk_k00")
            for k in range(KT):
                nc.tensor.matmul(
                    ps_q[:], lhsT=wqs[k][:, 0:128], rhs=xts[k][:, 0:512],
                    start=(k == 0), stop=(k == KT - 1),
                )
                nc.tensor.matmul(
                    ps_k[:], lhsT=wks[k][:, 0:128], rhs=xts[k][:, 0:512],
                    start=(k == 0), stop=(k == KT - 1),
                )
            nc.vector.tensor_add(
                qts[0][:, 0:512], ps_q[:], bq_s[:, 0:1].to_broadcast((128, 512))
            )
            nc.vector.tensor_add(
                kts[0][:, 0:512], ps_k[:], bk_s[:, 0:1].to_broadcast((128, 512))
            )

            # S0 (hp0, nq0): v chains JIT per step (DMA-paced); k0 n=1..3 and
            # q0 n=1 burst in when their x column slices land.
            fill0 = {m: [lambda m=m: v_group(m)] for m in range(MT)}
            fill0[3] = [lambda: qk_burst(0, "k", 1)] + fill0[3]
            fill0[6] = [lambda: qk_burst(0, "k", 2)] + fill0[6]
            fill0[9] = [lambda: qk_burst(0, "k", 3)] + fill0[9]
            fill0[11] = [lambda: qk_burst(0, "q", 1)] + fill0[11]
            pending = attn_slice(0, 0, fill0)

            # S1-S3 (hp0, nq1-3): remaining projections as woven 3-chunk
            # chains + previous slice's normalization pipeline.
            # Two chains per slice here; the rest ride in S4, which carries
            # no output-projection work and has the most PE slack.
            plans = {
                1: [(0, "q", 2), (1, "k", 0)],
                2: [(0, "q", 3), (1, "k", 1)],
                3: [(1, "q", 0), (1, "q", 1)],
            }
            for nq in range(1, NQ):
                fd = merge(
                    norm_fill(pending),
                    chain_fill(plans[nq], (2, 9)),
                )
                pending = attn_slice(0, nq, fd)

            # S4 (hp1, nq0): last q chain + norm(S3).
            fd = merge(
                norm_fill(pending),
                chain_fill(
                    [(1, "k", 2), (1, "k", 3), (1, "q", 2), (1, "q", 3)],
                    (1, 4, 9, 12),
                ),
            )
            pending = attn_slice(1, 0, fd)

            # S5-S7 (hp1, nq1-3): previous slice's normalization, then that
            # query slice's 8 output-projection groups one per step.
            for nq in range(1, NQ):
                fd = norm_fill(pending)
                for mo in range(OUT // 128):
                    fd.setdefault(8 + mo, []).append(
                        lambda n=nq - 1, mo=mo: out_proj_group(n, mo)
                    )
                pending = attn_slice(1, nq, fd)

            # Tail: last normalization + last query slice's projection, with
            # no-dep dummy matmuls woven in so HAM never sees a >3.4us PE
            # idle gap (a cold tail ran at half clock in earlier traces).
            def warm(k=2):
                for _ in range(k):
                    wps = psp.tile([128, 1024], FP32, tag="S", name="warm")
                    nc.tensor.matmul(
                        wps[:, 0:512], lhsT=wu[:, 0:128], rhs=wu[:], start=True, stop=True
                    )

            # Tail normalization goes back through ACT ln/exp: here LATENCY
            # matters (nothing left to overlap) and the two ACT ops beat the
            # serial single-partition DVE Newton chain; the natural_log_exp
            # table set is already resident. ACT produces +1/d, so the
            # broadcast uses the +1 ones vector.
            p1, _seed_t, _newt_t, _p2a, _p2b = pending
            p1()
            warm(3)
            hp_t, nq_t = 1, NQ - 1
            cs_t = p1.st["cs"]
            ln_t = work.tile([1, 1024], FP32, tag="lnt", name="ln_tail")
            nc.scalar.activation(
                ln_t[:], cs_t[64:65, :], mybir.ActivationFunctionType.Ln
            )
            warm(3)
            rr_t = work.tile([1, 1024], FP16, tag="rrt", name="rr_tail")
            nc.scalar.activation(
                rr_t[:], ln_t[:], mybir.ActivationFunctionType.Exp, scale=-1.0
            )
            warm(3)
            for a in range(2):
                bc = pop.tile([128, 512], FP32, tag="po", name=f"bct{a}")
                nc.tensor.matmul(
                    bc[0:64, :],
                    lhsT=ones_p[:],
                    rhs=rr_t[0:1, 512 * a : 512 * a + 512],
                    start=True,
                    stop=True,
                )
                nc.vector.tensor_mul(
                    cns[hp_t][64 * a : 64 * a + 64, nq_t * 512 : (nq_t + 1) * 512],
                    cs_t[0:64, 512 * a : 512 * a + 512],
                    bc[0:64, :],
                )
            for mo in range(OUT // 128):
                out_proj_group(NQ - 1, mo, pool=(psp if mo % 2 else None))

    _split_waits(nc)
    return nc


_PROGRAM = None


def _get_program():
    global _PROGRAM
    if _PROGRAM is None:
        _PROGRAM = build_program()
    return _PROGRAM


def _shard_inputs(x, Wq, bq, Wk, bk, Wv, bv, Wo, bo):
    f16 = np.float16
    in_maps = []
    for c in range(NCORES):
        b = c // 4
        g = c % 4
        hs = slice(g * HPC, (g + 1) * HPC)

        xTc = np.ascontiguousarray(x[b].T).astype(f16)  # [D, S]
        wq_c = np.ascontiguousarray(Wq[hs].transpose(1, 0, 2).reshape(D, E)).astype(f16)
        wk_c = np.ascontiguousarray(Wk[hs].transpose(1, 0, 2).reshape(D, E)).astype(f16)
        wv_c = np.zeros((D, EV), dtype=np.float32)
        mv_c = np.zeros((1, EV), dtype=np.float32)
        for h in range(HPC):
            wv_c[:, h * 65 : h * 65 + 64] = Wv[hs][h]
            mv_c[0, h * 65 : h * 65 + 64] = bv[hs][h]
            mv_c[0, h * 65 + 64] = 1.0
        wo_c = np.ascontiguousarray(Wo[g * E : (g + 1) * E, :]).astype(f16)
        bq_c = np.ascontiguousarray(bq[hs].reshape(E // 128, 128).T).astype(np.float32)
        bk_c = np.ascontiguousarray(bk[hs].reshape(E // 128, 128).T).astype(np.float32)
        bo_c = np.ascontiguousarray(
            (bo.astype(np.float32) * 0.25).reshape(OUT // 128, 128).T
        ).astype(np.float32)

        in_maps.append(
            {
                "xT": xTc,
                "wq": wq_c,
                "wk": wk_c,
                "wv": wv_c.astype(f16),
                "wo": wo_c,
                "bq": bq_c,
                "bk": bk_c,
                "maskv": np.ascontiguousarray(
                    np.broadcast_to(mv_c, (128, EV))
                ).astype(np.float32),
                "bo4": bo_c,
            }
        )
    return in_maps


def kernel(x, Wq, bq, Wk, bk, Wv, bv, Wo, bo, _trace=False, _result_box=None):
    in_maps = _shard_inputs(
        np.asarray(x, np.float32),
        np.asarray(Wq, np.float32),
        np.asarray(bq, np.float32),
        np.asarray(Wk, np.float32),
        np.asarray(bk, np.float32),
        np.asarray(Wv, np.float32),
        np.asarray(bv, np.float32),
        np.asarray(Wo, np.float32),
        np.asarray(bo, np.float32),
    )
    nc = _get_program()
    res = run_bass_kernel_spmd(nc, in_maps, list(range(NCORES)), trace=_trace)
    if _result_box is not None:
        _result_box.append(res)

    out = np.empty((B, S, OUT), dtype=np.float32)
    for b in range(B):
        acc = res.results[4 * b]["outT"].astype(np.float32)
        for g in range(1, 4):
            acc = acc + res.results[4 * b + g]["outT"].astype(np.float32)
        out[b] = acc.T
    return out


# revision 52
# speedup vs baseline: 1.1772x; 1.1772x over previous
"""Multi-head attention (B=2, S=2048, D=1024, H=16, dh=64) on 8 Trainium2 cores.

Sharding: head-tensor-parallel x batch. Core c owns batch b=c//4 and heads
4*(c%4)..4*(c%4)+3 (256 of the 1024 ctx dims). Each core computes its heads'
Q/K/V projections, attention, and a partial output projection against its
256 rows of Wo (+ bo/4 so the 4 partials per batch sum to one bo). The host
unshard step sums the 4 partial outputs per batch (the tensor-parallel
all-reduce), done at gather time.

Per-core kernel (fp16 matmul operands, fp32 PSUM accumulation):
  qT/kT [256e, 2048t] = W.T @ x.T computed directly in transposed form so
  scores^T [kt, qt] = (kT slice).T @ (qT slice) needs no on-device transpose.
  Head pairs are row-packed (heads at array rows 0-63 / 64-127) so the two
  K=64 score matmuls of a pair run concurrently via tile_position row groups.
  exp runs on ScalarE with the 1/sqrt(dh) scale folded in. A@V uses a
  stationary operand [V | 1] (ones column injected by the eviction mask-add)
  so the softmax denominator falls out of the same matmul. The denominator
  reciprocal is 1/s = exp(-ln(s)) in two ScalarE ops (same table set as the
  scores exp), woven into the next slice behind queued score exps; the
  1/rowsum row is broadcast across partitions with a K=1 fp16 matmul.

Schedule: DMA order is arranged so the first exp fires ~7us in (wq, wk, x
column-slice 0 first; v/k/q projections stream in JIT as later x column
slices land). Projection/output-projection matmul chains are chopped into
2-3 matmul chunks woven between attention m-steps so the exp cadence never
breaks. Each attn1 slice carries the previous slice's normalization and one
query-slice of output projection; only the last q-slice's projection remains
for the tail.
"""

import numpy as np

import bass_rust
import concourse.bass as bass
import concourse.mybir as mybir
import concourse.tile as tile
from concourse.bass_utils import run_bass_kernel_spmd

B = 2
S = 2048
D = 1024
H = 16
DH = 64
OUT = 1024
NCORES = 8
HPC = H // 4  # heads per core = 4
E = HPC * DH  # 256 ctx dims per core
EV = HPC * (DH + 1)  # 260: v with interleaved ones columns

FP16 = mybir.dt.float16  # fp16: same PE rate as bf16, 10-bit mantissa
FP32 = mybir.dt.float32
F32R = mybir.dt.float32r
I32 = mybir.dt.int32
ALU = mybir.AluOpType

SCALE = 1.0 / float(np.sqrt(DH))
# Reciprocal bit-trick seed: bitcast(~i + RECIP_C) == -(~5%-accurate 1/d)
# for positive d (Schraudolph constant with the sign bit folded in); one
# Newton step squares the error to ~2.6e-3, well under the accuracy gate.
RECIP_C = int(np.int32(np.uint32((0x7EF311C3 + 0x80000000 + 1) & 0xFFFFFFFF)))

KT = D // 128  # 8 k-tiles for projections
MT = S // 128  # 16 key-token tiles
NQ = S // 512  # 4 query slices of 512


def _split_waits(nc, maxw=1):
    """This container's walrus rejects instructions carrying more than one
    semaphore wait ("Too many sync wait commands"); hoist extras onto
    standalone same-engine nops, preserving per-engine program order."""
    for bb in nc.main_func.blocks:
        new_il = []
        for inst in bb.instructions:
            si = inst.sync_info
            if si is not None and si.on_wait and len(si.on_wait) > maxw:
                waits = list(si.on_wait)
                for j, w in enumerate(waits[:-maxw]):
                    nop = mybir.InstNoOp(
                        name=f"{inst.name}-ws{j}", ins=[], outs=[], engine=inst.engine
                    )
                    nop.sync_info = bass_rust.SyncInfo(on_wait=[w], on_update=[])
                    new_il.append(nop)
                inst.sync_info = bass_rust.SyncInfo(
                    on_wait=waits[-maxw:], on_update=list(si.on_update)
                )
            new_il.append(inst)
        bb.instructions = new_il


def build_program():
    nc = bass.Bass()

    xT = nc.declare_dram_parameter("xT", [D, S], FP16, isOutput=False)
    wq = nc.declare_dram_parameter("wq", [D, E], FP16, isOutput=False)
    wk = nc.declare_dram_parameter("wk", [D, E], FP16, isOutput=False)
    wv = nc.declare_dram_parameter("wv", [D, EV], FP16, isOutput=False)
    wo = nc.declare_dram_parameter("wo", [E, OUT], FP16, isOutput=False)
    bqp = nc.declare_dram_parameter("bq", [128, E // 128], FP32, isOutput=False)
    bkp = nc.declare_dram_parameter("bk", [128, E // 128], FP32, isOutput=False)
    mvp = nc.declare_dram_parameter("maskv", [128, EV], FP32, isOutput=False)
    bop = nc.declare_dram_parameter("bo4", [128, OUT // 128], FP32, isOutput=False)
    outT = nc.declare_dram_parameter("outT", [OUT, S], FP16, isOutput=True)

    with tile.TileContext(nc) as tc:
        with (
            tc.tile_pool(name="w", bufs=1) as wpool,
            tc.tile_pool(name="work", bufs=3) as work,
            tc.tile_pool(name="cnp", bufs=1) as cnpool,
            tc.tile_pool(name="ps", bufs=2, space="PSUM") as psp,
            tc.tile_pool(name="ctxps", bufs=2, space="PSUM") as ctxp,
            tc.tile_pool(name="pop", bufs=2, space="PSUM") as pop,
        ):
            # ---- persistent SBUF residents ----
            xts = [wpool.tile([128, S], FP16, tag=f"xt{k}", name=f"xt{k}") for k in range(KT)]
            wqs = [wpool.tile([128, E], FP16, tag=f"wq{k}", name=f"wq{k}") for k in range(KT)]
            wks = [wpool.tile([128, E], FP16, tag=f"wk{k}", name=f"wk{k}") for k in range(KT)]
            wvs = [wpool.tile([128, EV], FP16, tag=f"wv{k}", name=f"wv{k}") for k in range(KT)]
            wos = [wpool.tile([128, OUT], FP16, tag=f"wo{k}", name=f"wo{k}") for k in range(2)]
            bq_s = wpool.tile([128, E // 128], FP32, tag="bq")
            bk_s = wpool.tile([128, E // 128], FP32, tag="bk")
            mv_s = wpool.tile([128, EV], FP32, tag="mv")
            bo_s = wpool.tile([128, OUT // 128], FP32, tag="bo")
            ones_f = wpool.tile([1, 64], FP16, tag="ones_f")
            ones_p = wpool.tile([1, 64], FP16, tag="ones_p")
            qts = [wpool.tile([128, S], FP16, tag=f"qt{m}", name=f"qt{m}") for m in range(2)]
            kts = [wpool.tile([128, S], FP16, tag=f"kt{m}", name=f"kt{m}") for m in range(2)]
            vts = [wpool.tile([128, EV], FP16, tag=f"vt{m}", name=f"vt{m}") for m in range(MT)]
            cns = [cnpool.tile([128, S], FP16, tag=f"cn{m}", name=f"cn{m}") for m in range(2)]

            # DMA issue costs ~650ns each and is serial per engine queue, so
            # the critical first transfers (wq, wk, x column-slice 0) are
            # spread across the three DMA-capable queues (Sync, GpSimd,
            # Scalar) to issue in parallel. Scalar gets only wk so the exp
            # table load / first exp aren't queued behind DMA issues.
            for k in range(KT):
                nc.gpsimd.dma_start(out=wqs[k][:], in_=wq[k * 128 : (k + 1) * 128, :])
            for k in range(KT):
                nc.scalar.dma_start(out=wks[k][:], in_=wk[k * 128 : (k + 1) * 128, :])
            for k in range(KT):
                nc.sync.dma_start(
                    out=xts[k][:, 0:512], in_=xT[k * 128 : (k + 1) * 128, 0:512]
                )
            nc.sync.dma_start(out=bq_s[:], in_=bqp[:])
            nc.sync.dma_start(out=bk_s[:], in_=bkp[:])
            for k in range(KT):
                nc.sync.dma_start(
                    out=xts[k][:, 512:1024], in_=xT[k * 128 : (k + 1) * 128, 512:1024]
                )
            for k in range(KT):
                nc.gpsimd.dma_start(out=wvs[k][:], in_=wv[k * 128 : (k + 1) * 128, :])
            nc.gpsimd.dma_start(out=mv_s[:], in_=mvp[:])
            for k in range(KT):
                nc.gpsimd.dma_start(
                    out=xts[k][:, 1024:1536], in_=xT[k * 128 : (k + 1) * 128, 1024:1536]
                )
            for k in range(KT):
                nc.sync.dma_start(
                    out=xts[k][:, 1536:2048], in_=xT[k * 128 : (k + 1) * 128, 1536:2048]
                )
            for k in range(2):
                nc.gpsimd.dma_start(out=wos[k][:], in_=wo[k * 128 : (k + 1) * 128, :])
            nc.gpsimd.dma_start(out=bo_s[:], in_=bop[:])
            nc.vector.memset(ones_f[:], -1.0)
            nc.vector.memset(ones_p[:], 1.0)

            # Warm the PE clock (HAM un-throttles after ~3.4us sustained)
            # while the first DMAs stream in: no-dependency dummy matmuls.
            wu = wpool.tile([128, 512], FP16, tag="wu")
            nc.vector.memset(wu[:], 0.0)
            wups = psp.tile([128, 1024], FP32, tag="S", name="wups")
            for i in range(16):
                nc.tensor.matmul(
                    wups[:, 0:512], lhsT=wu[:, 0:128], rhs=wu[:], start=True, stop=True
                )

            # ---- projection chain helpers ----
            def qk_burst(hp, which, n):
                """Full 8-matmul projection group (prologue / slice-0 use)."""
                w_s, dst, bias = (
                    (wqs, qts, bq_s) if which == "q" else (wks, kts, bk_s)
                )
                ps = pop.tile([128, 512], FP32, tag="po", name=f"qk_{which}{hp}{n}")
                for k in range(KT):
                    nc.tensor.matmul(
                        ps[:],
                        lhsT=w_s[k][:, hp * 128 : (hp + 1) * 128],
                        rhs=xts[k][:, n * 512 : (n + 1) * 512],
                        start=(k == 0),
                        stop=(k == KT - 1),
                    )
                nc.vector.tensor_add(
                    dst[hp][:, n * 512 : (n + 1) * 512],
                    ps[:],
                    bias[:, hp : hp + 1].to_broadcast((128, 512)),
                )

            def qk_chunks(hp, which, n):
                """The same group as 3 thunks (3+3+2 matmuls) to weave between
                attention m-steps. The accumulator PSUM tile persists across
                chunks (pop ring, bufs=1 serializes chains)."""
                w_s, dst, bias = (
                    (wqs, qts, bq_s) if which == "q" else (wks, kts, bk_s)
                )
                state = {}

                def chunk(ks, first, last):
                    def t():
                        if first:
                            state["ps"] = pop.tile(
                                [128, 512], FP32, tag="po", name=f"qkc_{which}{hp}{n}"
                            )
                        ps = state["ps"]
                        for k in ks:
                            nc.tensor.matmul(
                                ps[:],
                                lhsT=w_s[k][:, hp * 128 : (hp + 1) * 128],
                                rhs=xts[k][:, n * 512 : (n + 1) * 512],
                                start=(k == 0),
                                stop=(k == KT - 1),
                            )
                        if last:
                            nc.vector.tensor_add(
                                dst[hp][:, n * 512 : (n + 1) * 512],
                                ps[:],
                                bias[:, hp : hp + 1].to_broadcast((128, 512)),
                            )
                    return t

                return [
                    chunk(range(0, 3), True, False),
                    chunk(range(3, 6), False, False),
                    chunk(range(6, 8), False, True),
                ]

            def v_group(m):
                """v_ext rows m*128..m*128+127 (token-major); the ones columns
                (and bv) are injected by the mask-add eviction, so no bias
                matmul is needed."""
                ps = pop.tile([128, 512], FP32, tag="po", name=f"psv{m}")
                for k in range(KT):
                    nc.tensor.matmul(
                        ps[:, :EV],
                        lhsT=xts[k][:, m * 128 : (m + 1) * 128],
                        rhs=wvs[k][:],
                        start=(k == 0),
                        stop=(k == KT - 1),
                    )
                nc.vector.tensor_add(vts[m][:], ps[:, :EV], mv_s[:])

            def norm_thunks(ctx_a, ctx_b, hp, nq):
                """The deferred normalization of a finished slice as 5 thunks:
                [p1-evict (DVE), ln (ACT), 1/x exp (ACT), head-a broadcast+
                scale, head-b broadcast+scale]. Spreading them mid-next-slice
                keeps the ACT recip from stalling the score-exp cadence."""
                st = {}

                def p1():
                    st["cs"] = work.tile([65, 1024], FP32, tag="cs", bufs=3, name="cs_ab")
                    nc.vector.tensor_copy(st["cs"][:, 0:512], ctx_a[:])
                    nc.vector.tensor_copy(st["cs"][:, 512:1024], ctx_b[:])
                    # Denominator row replicated to a partition-0 tile (PSUM
                    # source, so the partition remap is legal): the DVE
                    # Newton ops need SBUF operands at matching bases.
                    st["dd"] = work.tile([1, 1024], FP32, tag="dd", bufs=3, name="dd_ab")
                    nc.vector.tensor_copy(st["dd"][0:1, 0:512], ctx_a[64:65, :])
                    nc.vector.tensor_copy(st["dd"][0:1, 512:1024], ctx_b[64:65, :])

                def seed_t():
                    # z = bitcast(~i + C) ~= -1/d, entirely on DVE: the
                    # pacing ACT engine never runs the reciprocal.
                    st["z"] = work.tile([1, 1024], FP32, tag="zz", bufs=3, name="z_ab")
                    zn = work.tile([1, 1024], FP32, tag="zn", bufs=3, name="zn_ab")
                    nc.vector.tensor_scalar(
                        zn[:].bitcast(I32), st["dd"][:].bitcast(I32),
                        0, None, ALU.bitwise_not,
                    )
                    nc.vector.tensor_scalar(
                        st["z"][:].bitcast(I32), zn[:].bitcast(I32),
                        RECIP_C, None, ALU.add,
                    )

                def newt_t():
                    # One Newton step straight to fp16: rr = (d*z + 2)*z,
                    # still -1/d; the -1 ones vector in the broadcast matmul
                    # flips the sign back.
                    tt = work.tile([1, 1024], FP32, tag="tt", bufs=3, name="t_ab")
                    st["rr"] = work.tile([1, 1024], FP16, tag="rr", bufs=3, name="rr_ab")
                    nc.vector.tensor_mul(tt[:], st["dd"][:], st["z"][:])
                    nc.vector.scalar_tensor_tensor(
                        st["rr"][:], tt[:], 2.0, st["z"][:],
                        op0=ALU.add, op1=ALU.mult,
                    )

                def p2(a):
                    def t():
                        bc = pop.tile([128, 512], FP32, tag="po", name=f"bc{hp}{a}{nq}")
                        nc.tensor.matmul(
                            bc[0:64, :],
                            lhsT=ones_f[:],
                            rhs=st["rr"][0:1, 512 * a : 512 * a + 512],
                            start=True,
                            stop=True,
                        )
                        nc.vector.tensor_mul(
                            cns[hp][64 * a : 64 * a + 64, nq * 512 : (nq + 1) * 512],
                            st["cs"][0:64, 512 * a : 512 * a + 512],
                            bc[0:64, :],
                        )
                    return t

                p1.st = st  # the tail reads the cs tile out of the closure
                return [p1, seed_t, newt_t, p2(0), p2(1)]

            def out_proj_group(n, mo, pool=None):
                """One [128,512] tile of the partial out^T for query slice n.
                Output DMA issues alternate between the Sync and GpSimd
                queues (both idle here) so the tail's last issues aren't
                serialized on one queue. `pool` lets the tail alternate PSUM
                rings so back-to-back groups pipeline instead of serializing
                on the single pop buffer."""
                ps = (pool or pop).tile(
                    [128, 512], FP32, tag="S" if pool is not None else "po",
                    name=f"op{n}{mo}",
                )
                for k in range(2):
                    nc.tensor.matmul(
                        ps[:],
                        lhsT=wos[k][:, mo * 128 : (mo + 1) * 128],
                        rhs=cns[k][:, n * 512 : (n + 1) * 512],
                        start=(k == 0),
                        stop=(k == 1),
                    )
                ot = work.tile([128, 512], FP16, tag="ot")
                nc.vector.tensor_scalar_add(ot[:], ps[:], bo_s[:, mo : mo + 1])
                eng = nc.sync if mo % 2 == 0 else nc.gpsimd
                eng.dma_start(
                    out=outT[mo * 128 : (mo + 1) * 128, n * 512 : (n + 1) * 512],
                    in_=ot[:],
                )

            def attn_slice(hp, nq, fillers):
                """One query slice (512) of attention for head pair hp.
                fillers: dict m -> list of thunks emitted just before step m.
                Returns (cs, rr, hp, nq) for the deferred normalization."""
                ctx_a = ctxp.tile([65, 512], FP32, tag="ctx", name="ctx_a")
                ctx_b = ctxp.tile([65, 512], FP32, tag="ctx", name="ctx_b")
                for m in range(MT):
                    sps = psp.tile([128, 1024], FP32, tag="S", name="sps")
                    nc.tensor.matmul(
                        sps[:, 0:512],
                        lhsT=kts[hp][0:64, m * 128 : (m + 1) * 128],
                        rhs=qts[hp][0:64, nq * 512 : (nq + 1) * 512],
                        start=True,
                        stop=True,
                    )
                    nc.tensor.matmul(
                        sps[:, 512:1024],
                        lhsT=kts[hp][64:128, m * 128 : (m + 1) * 128],
                        rhs=qts[hp][64:128, nq * 512 : (nq + 1) * 512],
                        start=True,
                        stop=True,
                    )
                    ee = work.tile([128, 1024], FP16, tag="E", bufs=4)
                    nc.scalar.activation(
                        ee[:], sps[:], mybir.ActivationFunctionType.Exp, scale=SCALE
                    )
                    # Fillers go BETWEEN the scores pair and the A@V pair on
                    # the PE queue: the next step's scores (and so the exp
                    # cadence) never queue behind filler work; only the AVs
                    # slip, absorbed by the ee ring.
                    for f in fillers.get(m, ()):
                        f()
                    ha = 2 * hp
                    nc.tensor.matmul(
                        ctx_a[:],
                        lhsT=vts[m][:, ha * 65 : ha * 65 + 65],
                        rhs=ee[:, 0:512],
                        start=(m == 0),
                        stop=(m == MT - 1),
                    )
                    nc.tensor.matmul(
                        ctx_b[:],
                        lhsT=vts[m][:, (ha + 1) * 65 : (ha + 1) * 65 + 65],
                        rhs=ee[:, 512:1024],
                        start=(m == 0),
                        stop=(m == MT - 1),
                    )
                return norm_thunks(ctx_a, ctx_b, hp, nq)

            def merge(*fds):
                out = {}
                for fd in fds:
                    for k, v in fd.items():
                        out[k] = out.get(k, []) + list(v)
                return out

            def norm_fill(pending, at=(0, 1, 2, 6, 7)):
                """Weave the previous slice's normalization pipeline into
                this slice: DVE evict at 0, ACT ln/exp at 4/5 (behind a few
                queued score exps so the ACT never starves the cadence),
                broadcasts at 6/7."""
                return {s: [t] for s, t in zip(at, pending)}

            def chain_fill(chains, starts):
                """Place each chain's 3 chunks at steps s, s+1, s+2."""
                fd = {}
                for (hp, which, n), s in zip(chains, starts):
                    for i, t in enumerate(qk_chunks(hp, which, n)):
                        fd.setdefault(s + i, []).append(t)
                return fd

            # ---- emission schedule ----
            # Prologue: q0 n=0 and k0 n=0, k-interleaved so both chains ride
            # the same DMA wave (each x k-tile feeds both the q and k mm).
            ps_q = pop.tile([128, 512], FP32, tag="po", name="qk_q00")
            ps_k = pop.tile([128, 512], FP32, tag="po", name="qk_k00")
            for k in range(KT):
                nc.tensor.matmul(
                    ps_q[:], lhsT=wqs[k][:, 0:128], rhs=xts[k][:, 0:512],
                    start=(k == 0), stop=(k == KT - 1),
                )
                nc.tensor.matmul(
                    ps_k[:], lhsT=wks[k][:, 0:128], rhs=xts[k][:, 0:512],
                    start=(k == 0), stop=(k == KT - 1),
                )
            nc.vector.tensor_add(
                qts[0][:, 0:512], ps_q[:], bq_s[:, 0:1].to_broadcast((128, 512))
            )
            nc.vector.tensor_add(
                kts[0][:, 0:512], ps_k[:], bk_s[:, 0:1].to_broadcast((128, 512))
            )

            # S0 (hp0, nq0): v chains JIT per step (DMA-paced); k0 n=1..3 and
            # q0 n=1 burst in when their x column slices land.
            fill0 = {m: [lambda m=m: v_group(m)] for m in range(MT)}
            fill0[3] = [lambda: qk_burst(0, "k", 1)] + fill0[3]
            fill0[6] = [lambda: qk_burst(0, "k", 2)] + fill0[6]
            fill0[9] = [lambda: qk_burst(0, "k", 3)] + fill0[9]
            fill0[11] = [lambda: qk_burst(0, "q", 1)] + fill0[11]
            pending = attn_slice(0, 0, fill0)

            # S1-S3 (hp0, nq1-3): remaining projections as woven 3-chunk
            # chains + previous slice's normalization pipeline.
            # Two chains per slice here; the rest ride in S4, which carries
            # no output-projection work and has the most PE slack.
            plans = {
                1: [(0, "q", 2), (1, "k", 0)],
                2: [(0, "q", 3), (1, "k", 1)],
                3: [(1, "q", 0), (1, "q", 1)],
            }
            for nq in range(1, NQ):
                fd = merge(
                    norm_fill(pending),
                    chain_fill(plans[nq], (2, 9)),
                )
                pending = attn_slice(0, nq, fd)

            # S4 (hp1, nq0): last q chain + norm(S3).
            fd = merge(
                norm_fill(pending),
                chain_fill(
                    [(1, "k", 2), (1, "k", 3), (1, "q", 2), (1, "q", 3)],
                    (1, 4, 9, 12),
                ),
            )
            pending = attn_slice(1, 0, fd)

            # S5-S7 (hp1, nq1-3): previous slice's normalization, then that
            # query slice's 8 output-projection groups one per step.
            for nq in range(1, NQ):
                fd = norm_fill(pending)
                for mo in range(OUT // 128):
                    fd.setdefault(8 + mo, []).append(
                        lambda n=nq - 1, mo=mo: out_proj_group(n, mo)
                    )
                pending = attn_slice(1, nq, fd)

            # Tail: last normalization + last query slice's projection, with
            # no-dep dummy matmuls woven in so HAM never sees a >3.4us PE
            # idle gap (a cold tail ran at half clock in earlier traces).
            def warm(k=2):
                for _ in range(k):
                    wps = psp.tile([128, 1024], FP32, tag="S", name="warm")
                    nc.tensor.matmul(
                        wps[:, 0:512], lhsT=wu[:, 0:128], rhs=wu[:], start=True, stop=True
                    )

            # Tail normalization goes back through ACT ln/exp: here LATENCY
            # matters (nothing left to overlap) and the two ACT ops beat the
            # serial single-partition DVE Newton chain; the natural_log_exp
            # table set is already resident. ACT produces +1/d, so the
            # broadcast uses the +1 ones vector.
            p1, _seed_t, _newt_t, _p2a, _p2b = pending
            p1()
            warm(3)
            hp_t, nq_t = 1, NQ - 1
            cs_t = p1.st["cs"]
            ln_t = work.tile([1, 1024], FP32, tag="lnt", name="ln_tail")
            nc.scalar.activation(
                ln_t[:], cs_t[64:65, :], mybir.ActivationFunctionType.Ln
            )
            warm(3)
            rr_t = work.tile([1, 1024], FP16, tag="rrt", name="rr_tail")
            nc.scalar.activation(
                rr_t[:], ln_t[:], mybir.ActivationFunctionType.Exp, scale=-1.0
            )
            warm(3)
            for a in range(2):
                bc = pop.tile([128, 512], FP32, tag="po", name=f"bct{a}")
                nc.tensor.matmul(
                    bc[0:64, :],
                    lhsT=ones_p[:],
                    rhs=rr_t[0:1, 512 * a : 512 * a + 512],
                    start=True,
                    stop=True,
                )
                nc.vector.tensor_mul(
                    cns[hp_t][64 * a : 64 * a + 64, nq_t * 512 : (nq_t + 1) * 512],
                    cs_t[0:64, 512 * a : 512 * a + 512],
                    bc[0:64, :],
                )
            for mo in range(OUT // 128):
                out_proj_group(NQ - 1, mo, pool=(psp if mo % 2 else None))

    _split_waits(nc)
    return nc


_PROGRAM = None


def _get_program():
    global _PROGRAM
    if _PROGRAM is None:
        _PROGRAM = build_program()
    return _PROGRAM


def _shard_inputs(x, Wq, bq, Wk, bk, Wv, bv, Wo, bo):
    f16 = np.float16
    in_maps = []
    for c in range(NCORES):
        b = c // 4
        g = c % 4
        hs = slice(g * HPC, (g + 1) * HPC)

        xTc = np.ascontiguousarray(x[b].T).astype(f16)  # [D, S]
        wq_c = np.ascontiguousarray(Wq[hs].transpose(1, 0, 2).reshape(D, E)).astype(f16)
        wk_c = np.ascontiguousarray(Wk[hs].transpose(1, 0, 2).reshape(D, E)).astype(f16)
        wv_c = np.zeros((D, EV), dtype=np.float32)
        mv_c = np.zeros((1, EV), dtype=np.float32)
        for h in range(HPC):
            wv_c[:, h * 65 : h * 65 + 64] = Wv[hs][h]
            mv_c[0, h * 65 : h * 65 + 64] = bv[hs][h]
            mv_c[0, h * 65 + 64] = 1.0
        wo_c = np.ascontiguousarray(Wo[g * E : (g + 1) * E, :]).astype(f16)
        bq_c = np.ascontiguousarray(bq[hs].reshape(E // 128, 128).T).astype(np.float32)
        bk_c = np.ascontiguousarray(bk[hs].reshape(E // 128, 128).T).astype(np.float32)
        bo_c = np.ascontiguousarray(
            (bo.astype(np.float32) * 0.25).reshape(OUT // 128, 128).T
        ).astype(np.float32)

        in_maps.append(
            {
                "xT": xTc,
                "wq": wq_c,
                "wk": wk_c,
                "wv": wv_c.astype(f16),
                "wo": wo_c,
                "bq": bq_c,
                "bk": bk_c,
                "maskv": np.ascontiguousarray(
                    np.broadcast_to(mv_c, (128, EV))
                ).astype(np.float32),
                "bo4": bo_c,
            }
        )
    return in_maps


def kernel(x, Wq, bq, Wk, bk, Wv, bv, Wo, bo, _trace=False, _result_box=None):
    in_maps = _shard_inputs(
        np.asarray(x, np.float32),
        np.asarray(Wq, np.float32),
        np.asarray(bq, np.float32),
        np.asarray(Wk, np.float32),
        np.asarray(bk, np.float32),
        np.asarray(Wv, np.float32),
        np.asarray(bv, np.float32),
        np.asarray(Wo, np.float32),
        np.asarray(bo, np.float32),
    )
    nc = _get_program()
    res = run_bass_kernel_spmd(nc, in_maps, list(range(NCORES)), trace=_trace)
    if _result_box is not None:
        _result_box.append(res)

    out = np.empty((B, S, OUT), dtype=np.float32)
    for b in range(B):
        acc = res.results[4 * b]["outT"].astype(np.float32)
        for g in range(1, 4):
            acc = acc + res.results[4 * b + g]["outT"].astype(np.float32)
        out[b] = acc.T
    return out


# revision 53
# speedup vs baseline: 1.1818x; 1.0039x over previous
"""Multi-head attention (B=2, S=2048, D=1024, H=16, dh=64) on 8 Trainium2 cores.

Sharding: head-tensor-parallel x batch. Core c owns batch b=c//4 and heads
4*(c%4)..4*(c%4)+3 (256 of the 1024 ctx dims). Each core computes its heads'
Q/K/V projections, attention, and a partial output projection against its
256 rows of Wo (+ bo/4 so the 4 partials per batch sum to one bo). The host
unshard step sums the 4 partial outputs per batch (the tensor-parallel
all-reduce), done at gather time.

Per-core kernel (fp16 matmul operands, fp32 PSUM accumulation):
  qT/kT [256e, 2048t] = W.T @ x.T computed directly in transposed form so
  scores^T [kt, qt] = (kT slice).T @ (qT slice) needs no on-device transpose.
  Head pairs are row-packed (heads at array rows 0-63 / 64-127) so the two
  K=64 score matmuls of a pair run concurrently via tile_position row groups.
  exp runs on ScalarE with the 1/sqrt(dh) scale folded in. A@V uses a
  stationary operand [V | 1] (ones column injected by the eviction mask-add)
  so the softmax denominator falls out of the same matmul. The denominator
  reciprocal is 1/s = exp(-ln(s)) in two ScalarE ops (same table set as the
  scores exp), woven into the next slice behind queued score exps; the
  1/rowsum row is broadcast across partitions with a K=1 fp16 matmul.

Schedule: DMA order is arranged so the first exp fires ~7us in (wq, wk, x
column-slice 0 first; v/k/q projections stream in JIT as later x column
slices land). Projection/output-projection matmul chains are chopped into
2-3 matmul chunks woven between attention m-steps so the exp cadence never
breaks. Each attn1 slice carries the previous slice's normalization and one
query-slice of output projection; only the last q-slice's projection remains
for the tail.
"""

import numpy as np

import bass_rust
import concourse.bass as bass
import concourse.mybir as mybir
import concourse.tile as tile
from concourse.bass_utils import run_bass_kernel_spmd

B = 2
S = 2048
D = 1024
H = 16
DH = 64
OUT = 1024
NCORES = 8
HPC = H // 4  # heads per core = 4
E = HPC * DH  # 256 ctx dims per core
EV = HPC * (DH + 1)  # 260: v with interleaved ones columns

FP16 = mybir.dt.float16  # fp16: same PE rate as bf16, 10-bit mantissa
FP32 = mybir.dt.float32
F32R = mybir.dt.float32r
I32 = mybir.dt.int32
ALU = mybir.AluOpType

SCALE = 1.0 / float(np.sqrt(DH))
# Reciprocal bit-trick seed: bitcast(~i + RECIP_C) == -(~5%-accurate 1/d)
# for positive d (Schraudolph constant with the sign bit folded in); one
# Newton step squares the error to ~2.6e-3, well under the accuracy gate.
RECIP_C = int(np.int32(np.uint32((0x7EF311C3 + 0x80000000 + 1) & 0xFFFFFFFF)))

KT = D // 128  # 8 k-tiles for projections
MT = S // 128  # 16 key-token tiles
NQ = S // 512  # 4 query slices of 512


def _split_waits(nc, maxw=1):
    """This container's walrus rejects instructions carrying more than one
    semaphore wait ("Too many sync wait commands"); hoist extras onto
    standalone same-engine nops, preserving per-engine program order."""
    for bb in nc.main_func.blocks:
        new_il = []
        for inst in bb.instructions:
            si = inst.sync_info
            if si is not None and si.on_wait and len(si.on_wait) > maxw:
                waits = list(si.on_wait)
                for j, w in enumerate(waits[:-maxw]):
                    nop = mybir.InstNoOp(
                        name=f"{inst.name}-ws{j}", ins=[], outs=[], engine=inst.engine
                    )
                    nop.sync_info = bass_rust.SyncInfo(on_wait=[w], on_update=[])
                    new_il.append(nop)
                inst.sync_info = bass_rust.SyncInfo(
                    on_wait=waits[-maxw:], on_update=list(si.on_update)
                )
            new_il.append(inst)
        bb.instructions = new_il


def build_program():
    nc = bass.Bass()

    xT = nc.declare_dram_parameter("xT", [D, S], FP16, isOutput=False)
    wq = nc.declare_dram_parameter("wq", [D, E], FP16, isOutput=False)
    wk = nc.declare_dram_parameter("wk", [D, E], FP16, isOutput=False)
    wv = nc.declare_dram_parameter("wv", [D, EV], FP16, isOutput=False)
    wo = nc.declare_dram_parameter("wo", [E, OUT], FP16, isOutput=False)
    bqp = nc.declare_dram_parameter("bq", [128, E // 128], FP32, isOutput=False)
    bkp = nc.declare_dram_parameter("bk", [128, E // 128], FP32, isOutput=False)
    mvp = nc.declare_dram_parameter("maskv", [128, EV], FP32, isOutput=False)
    bop = nc.declare_dram_parameter("bo4", [128, OUT // 128], FP32, isOutput=False)
    outT = nc.declare_dram_parameter("outT", [OUT, S], FP16, isOutput=True)

    with tile.TileContext(nc) as tc:
        with (
            tc.tile_pool(name="w", bufs=1) as wpool,
            tc.tile_pool(name="work", bufs=3) as work,
            tc.tile_pool(name="cnp", bufs=1) as cnpool,
            tc.tile_pool(name="ps", bufs=2, space="PSUM") as psp,
            tc.tile_pool(name="ctxps", bufs=2, space="PSUM") as ctxp,
            tc.tile_pool(name="pop", bufs=2, space="PSUM") as pop,
        ):
            # ---- persistent SBUF residents ----
            xts = [wpool.tile([128, S], FP16, tag=f"xt{k}", name=f"xt{k}") for k in range(KT)]
            wqs = [wpool.tile([128, E], FP16, tag=f"wq{k}", name=f"wq{k}") for k in range(KT)]
            wks = [wpool.tile([128, E], FP16, tag=f"wk{k}", name=f"wk{k}") for k in range(KT)]
            wvs = [wpool.tile([128, EV], FP16, tag=f"wv{k}", name=f"wv{k}") for k in range(KT)]
            wos = [wpool.tile([128, OUT], FP16, tag=f"wo{k}", name=f"wo{k}") for k in range(2)]
            bq_s = wpool.tile([128, E // 128], FP32, tag="bq")
            bk_s = wpool.tile([128, E // 128], FP32, tag="bk")
            mv_s = wpool.tile([128, EV], FP32, tag="mv")
            bo_s = wpool.tile([128, OUT // 128], FP32, tag="bo")
            ones_f = wpool.tile([1, 64], FP16, tag="ones_f")
            ones_p = wpool.tile([1, 64], FP16, tag="ones_p")
            qts = [wpool.tile([128, S], FP16, tag=f"qt{m}", name=f"qt{m}") for m in range(2)]
            kts = [wpool.tile([128, S], FP16, tag=f"kt{m}", name=f"kt{m}") for m in range(2)]
            vts = [wpool.tile([128, EV], FP16, tag=f"vt{m}", name=f"vt{m}") for m in range(MT)]
            cns = [cnpool.tile([128, S], FP16, tag=f"cn{m}", name=f"cn{m}") for m in range(2)]

            # DMA issue costs ~650ns each and is serial per engine queue, so
            # the critical first transfers (wq, wk, x column-slice 0) are
            # spread across the three DMA-capable queues (Sync, GpSimd,
            # Scalar) to issue in parallel. Scalar gets only wk so the exp
            # table load / first exp aren't queued behind DMA issues.
            for k in range(KT):
                nc.gpsimd.dma_start(out=wqs[k][:], in_=wq[k * 128 : (k + 1) * 128, :])
            for k in range(KT):
                nc.scalar.dma_start(out=wks[k][:], in_=wk[k * 128 : (k + 1) * 128, :])
            for k in range(KT):
                nc.sync.dma_start(
                    out=xts[k][:, 0:512], in_=xT[k * 128 : (k + 1) * 128, 0:512]
                )
            nc.sync.dma_start(out=bq_s[:], in_=bqp[:])
            nc.sync.dma_start(out=bk_s[:], in_=bkp[:])
            for k in range(KT):
                nc.sync.dma_start(
                    out=xts[k][:, 512:1024], in_=xT[k * 128 : (k + 1) * 128, 512:1024]
                )
            for k in range(KT):
                nc.gpsimd.dma_start(out=wvs[k][:], in_=wv[k * 128 : (k + 1) * 128, :])
            nc.gpsimd.dma_start(out=mv_s[:], in_=mvp[:])
            for k in range(KT):
                nc.gpsimd.dma_start(
                    out=xts[k][:, 1024:1536], in_=xT[k * 128 : (k + 1) * 128, 1024:1536]
                )
            for k in range(KT):
                nc.sync.dma_start(
                    out=xts[k][:, 1536:2048], in_=xT[k * 128 : (k + 1) * 128, 1536:2048]
                )
            for k in range(2):
                nc.gpsimd.dma_start(out=wos[k][:], in_=wo[k * 128 : (k + 1) * 128, :])
            nc.gpsimd.dma_start(out=bo_s[:], in_=bop[:])
            nc.vector.memset(ones_f[:], -1.0)
            nc.vector.memset(ones_p[:], 1.0)

            # Warm the PE clock (HAM un-throttles after ~3.4us sustained)
            # while the first DMAs stream in: no-dependency dummy matmuls.
            wu = wpool.tile([128, 512], FP16, tag="wu")
            nc.vector.memset(wu[:], 0.0)
            wups = psp.tile([128, 1024], FP32, tag="S", name="wups")
            for i in range(16):
                nc.tensor.matmul(
                    wups[:, 0:512], lhsT=wu[:, 0:128], rhs=wu[:], start=True, stop=True
                )

            # ---- projection chain helpers ----
            def qk_burst(hp, which, n):
                """Full 8-matmul projection group (prologue / slice-0 use)."""
                w_s, dst, bias = (
                    (wqs, qts, bq_s) if which == "q" else (wks, kts, bk_s)
                )
                ps = pop.tile([128, 512], FP32, tag="po", name=f"qk_{which}{hp}{n}")
                for k in range(KT):
                    nc.tensor.matmul(
                        ps[:],
                        lhsT=w_s[k][:, hp * 128 : (hp + 1) * 128],
                        rhs=xts[k][:, n * 512 : (n + 1) * 512],
                        start=(k == 0),
                        stop=(k == KT - 1),
                    )
                nc.vector.tensor_add(
                    dst[hp][:, n * 512 : (n + 1) * 512],
                    ps[:],
                    bias[:, hp : hp + 1].to_broadcast((128, 512)),
                )

            def qk_chunks(hp, which, n):
                """The same group as 3 thunks (3+3+2 matmuls) to weave between
                attention m-steps. The accumulator PSUM tile persists across
                chunks (pop ring, bufs=1 serializes chains)."""
                w_s, dst, bias = (
                    (wqs, qts, bq_s) if which == "q" else (wks, kts, bk_s)
                )
                state = {}

                def chunk(ks, first, last):
                    def t():
                        if first:
                            state["ps"] = pop.tile(
                                [128, 512], FP32, tag="po", name=f"qkc_{which}{hp}{n}"
                            )
                        ps = state["ps"]
                        for k in ks:
                            nc.tensor.matmul(
                                ps[:],
                                lhsT=w_s[k][:, hp * 128 : (hp + 1) * 128],
                                rhs=xts[k][:, n * 512 : (n + 1) * 512],
                                start=(k == 0),
                                stop=(k == KT - 1),
                            )
                        if last:
                            nc.vector.tensor_add(
                                dst[hp][:, n * 512 : (n + 1) * 512],
                                ps[:],
                                bias[:, hp : hp + 1].to_broadcast((128, 512)),
                            )
                    return t

                return [
                    chunk(range(0, 3), True, False),
                    chunk(range(3, 6), False, False),
                    chunk(range(6, 8), False, True),
                ]

            def v_group(m):
                """v_ext rows m*128..m*128+127 (token-major); the ones columns
                (and bv) are injected by the mask-add eviction, so no bias
                matmul is needed."""
                ps = pop.tile([128, 512], FP32, tag="po", name=f"psv{m}")
                for k in range(KT):
                    nc.tensor.matmul(
                        ps[:, :EV],
                        lhsT=xts[k][:, m * 128 : (m + 1) * 128],
                        rhs=wvs[k][:],
                        start=(k == 0),
                        stop=(k == KT - 1),
                    )
                nc.vector.tensor_add(vts[m][:], ps[:, :EV], mv_s[:])

            def norm_thunks(ctx_a, ctx_b, hp, nq):
                """The deferred normalization of a finished slice as 5 thunks:
                [p1-evict (DVE), ln (ACT), 1/x exp (ACT), head-a broadcast+
                scale, head-b broadcast+scale]. Spreading them mid-next-slice
                keeps the ACT recip from stalling the score-exp cadence."""
                st = {}

                def p1():
                    st["cs"] = work.tile([65, 1024], FP32, tag="cs", bufs=3, name="cs_ab")
                    nc.vector.tensor_copy(st["cs"][:, 0:512], ctx_a[:])
                    nc.vector.tensor_copy(st["cs"][:, 512:1024], ctx_b[:])
                    # Denominator row replicated to a partition-0 tile (PSUM
                    # source, so the partition remap is legal): the DVE
                    # Newton ops need SBUF operands at matching bases.
                    st["dd"] = work.tile([1, 1024], FP32, tag="dd", bufs=3, name="dd_ab")
                    nc.vector.tensor_copy(st["dd"][0:1, 0:512], ctx_a[64:65, :])
                    nc.vector.tensor_copy(st["dd"][0:1, 512:1024], ctx_b[64:65, :])

                def seed_t():
                    # z = bitcast(~i + C) ~= -1/d, entirely on DVE: the
                    # pacing ACT engine never runs the reciprocal.
                    st["z"] = work.tile([1, 1024], FP32, tag="zz", bufs=3, name="z_ab")
                    zn = work.tile([1, 1024], FP32, tag="zn", bufs=3, name="zn_ab")
                    nc.vector.tensor_scalar(
                        zn[:].bitcast(I32), st["dd"][:].bitcast(I32),
                        0, None, ALU.bitwise_not,
                    )
                    nc.vector.tensor_scalar(
                        st["z"][:].bitcast(I32), zn[:].bitcast(I32),
                        RECIP_C, None, ALU.add,
                    )

                def newt_t():
                    # One Newton step straight to fp16: rr = (d*z + 2)*z,
                    # still -1/d; the -1 ones vector in the broadcast matmul
                    # flips the sign back.
                    tt = work.tile([1, 1024], FP32, tag="tt", bufs=3, name="t_ab")
                    st["rr"] = work.tile([1, 1024], FP16, tag="rr", bufs=3, name="rr_ab")
                    nc.vector.tensor_mul(tt[:], st["dd"][:], st["z"][:])
                    nc.vector.scalar_tensor_tensor(
                        st["rr"][:], tt[:], 2.0, st["z"][:],
                        op0=ALU.add, op1=ALU.mult,
                    )

                def p2(a):
                    def t():
                        bc = pop.tile([128, 512], FP32, tag="po", name=f"bc{hp}{a}{nq}")
                        nc.tensor.matmul(
                            bc[0:64, :],
                            lhsT=ones_f[:],
                            rhs=st["rr"][0:1, 512 * a : 512 * a + 512],
                            start=True,
                            stop=True,
                        )
                        nc.vector.tensor_mul(
                            cns[hp][64 * a : 64 * a + 64, nq * 512 : (nq + 1) * 512],
                            st["cs"][0:64, 512 * a : 512 * a + 512],
                            bc[0:64, :],
                        )
                    return t

                p1.st = st  # the tail reads the cs tile out of the closure
                return [p1, seed_t, newt_t, p2(0), p2(1)]

            def out_proj_group(n, mo, pool=None):
                """One [128,512] tile of the partial out^T for query slice n.
                Output DMA issues alternate between the Sync and GpSimd
                queues (both idle here) so the tail's last issues aren't
                serialized on one queue. `pool` lets the tail alternate PSUM
                rings so back-to-back groups pipeline instead of serializing
                on the single pop buffer."""
                ps = (pool or pop).tile(
                    [128, 512], FP32, tag="S" if pool is not None else "po",
                    name=f"op{n}{mo}",
                )
                for k in range(2):
                    nc.tensor.matmul(
                        ps[:],
                        lhsT=wos[k][:, mo * 128 : (mo + 1) * 128],
                        rhs=cns[k][:, n * 512 : (n + 1) * 512],
                        start=(k == 0),
                        stop=(k == 1),
                    )
                ot = work.tile([128, 512], FP16, tag="ot")
                nc.vector.tensor_scalar_add(ot[:], ps[:], bo_s[:, mo : mo + 1])
                eng = nc.sync if mo % 2 == 0 else nc.gpsimd
                eng.dma_start(
                    out=outT[mo * 128 : (mo + 1) * 128, n * 512 : (n + 1) * 512],
                    in_=ot[:],
                )

            def attn_slice(hp, nq, fillers):
                """One query slice (512) of attention for head pair hp.
                fillers: dict m -> list of thunks emitted just before step m.
                Returns (cs, rr, hp, nq) for the deferred normalization."""
                ctx_a = ctxp.tile([65, 512], FP32, tag="ctx", name="ctx_a")
                ctx_b = ctxp.tile([65, 512], FP32, tag="ctx", name="ctx_b")
                for m in range(MT):
                    sps = psp.tile([128, 1024], FP32, tag="S", name="sps")
                    nc.tensor.matmul(
                        sps[:, 0:512],
                        lhsT=kts[hp][0:64, m * 128 : (m + 1) * 128],
                        rhs=qts[hp][0:64, nq * 512 : (nq + 1) * 512],
                        start=True,
                        stop=True,
                    )
                    nc.tensor.matmul(
                        sps[:, 512:1024],
                        lhsT=kts[hp][64:128, m * 128 : (m + 1) * 128],
                        rhs=qts[hp][64:128, nq * 512 : (nq + 1) * 512],
                        start=True,
                        stop=True,
                    )
                    ee = work.tile([128, 1024], FP16, tag="E", bufs=4)
                    nc.scalar.activation(
                        ee[:], sps[:], mybir.ActivationFunctionType.Exp, scale=SCALE
                    )
                    # Fillers go BETWEEN the scores pair and the A@V pair on
                    # the PE queue: the next step's scores (and so the exp
                    # cadence) never queue behind filler work; only the AVs
                    # slip, absorbed by the ee ring.
                    for f in fillers.get(m, ()):
                        f()
                    ha = 2 * hp
                    nc.tensor.matmul(
                        ctx_a[:],
                        lhsT=vts[m][:, ha * 65 : ha * 65 + 65],
                        rhs=ee[:, 0:512],
                        start=(m == 0),
                        stop=(m == MT - 1),
                    )
                    nc.tensor.matmul(
                        ctx_b[:],
                        lhsT=vts[m][:, (ha + 1) * 65 : (ha + 1) * 65 + 65],
                        rhs=ee[:, 512:1024],
                        start=(m == 0),
                        stop=(m == MT - 1),
                    )
                return norm_thunks(ctx_a, ctx_b, hp, nq)

            def merge(*fds):
                out = {}
                for fd in fds:
                    for k, v in fd.items():
                        out[k] = out.get(k, []) + list(v)
                return out

            def norm_fill(pending, at=(0, 1, 2, 6, 7)):
                """Weave the previous slice's normalization pipeline into
                this slice: DVE evict at 0, ACT ln/exp at 4/5 (behind a few
                queued score exps so the ACT never starves the cadence),
                broadcasts at 6/7."""
                return {s: [t] for s, t in zip(at, pending)}

            def chain_fill(chains, starts):
                """Place each chain's 3 chunks at steps s, s+1, s+2."""
                fd = {}
                for (hp, which, n), s in zip(chains, starts):
                    for i, t in enumerate(qk_chunks(hp, which, n)):
                        fd.setdefault(s + i, []).append(t)
                return fd

            # ---- emission schedule ----
            # Prologue: q0 n=0 and k0 n=0, k-interleaved so both chains ride
            # the same DMA wave (each x k-tile feeds both the q and k mm).
            ps_q = pop.tile([128, 512], FP32, tag="po", name="qk_q00")
            ps_k = pop.tile([128, 512], FP32, tag="po", name="qk_k00")
            for k in range(KT):
                nc.tensor.matmul(
                    ps_q[:], lhsT=wqs[k][:, 0:128], rhs=xts[k][:, 0:512],
                    start=(k == 0), stop=(k == KT - 1),
                )
                nc.tensor.matmul(
                    ps_k[:], lhsT=wks[k][:, 0:128], rhs=xts[k][:, 0:512],
                    start=(k == 0), stop=(k == KT - 1),
                )
                # No-dep dummy fills the DMA-paced gap to the next k-tile so
                # HAM never re-throttles the PE clock mid-prologue (observed
                # at K=4/8 through the first chains and scores otherwise).
                nc.tensor.matmul(
                    wups[:, 0:512], lhsT=wu[:, 0:128], rhs=wu[:],
                    start=True, stop=True,
                )
            nc.vector.tensor_add(
                qts[0][:, 0:512], ps_q[:], bq_s[:, 0:1].to_broadcast((128, 512))
            )
            nc.vector.tensor_add(
                kts[0][:, 0:512], ps_k[:], bk_s[:, 0:1].to_broadcast((128, 512))
            )

            # S0 (hp0, nq0): v chains JIT per step (DMA-paced); k0 n=1..3 and
            # q0 n=1 burst in when their x column slices land.
            fill0 = {m: [lambda m=m: v_group(m)] for m in range(MT)}
            fill0[3] = [lambda: qk_burst(0, "k", 1)] + fill0[3]
            fill0[6] = [lambda: qk_burst(0, "k", 2)] + fill0[6]
            fill0[9] = [lambda: qk_burst(0, "k", 3)] + fill0[9]
            fill0[11] = [lambda: qk_burst(0, "q", 1)] + fill0[11]
            pending = attn_slice(0, 0, fill0)

            # S1-S3 (hp0, nq1-3): remaining projections as woven 3-chunk
            # chains + previous slice's normalization pipeline.
            # Two chains per slice here; the rest ride in S4, which carries
            # no output-projection work and has the most PE slack.
            plans = {
                1: [(0, "q", 2), (1, "k", 0)],
                2: [(0, "q", 3), (1, "k", 1)],
                3: [(1, "q", 0), (1, "q", 1)],
            }
            for nq in range(1, NQ):
                fd = merge(
                    norm_fill(pending),
                    chain_fill(plans[nq], (2, 9)),
                )
                pending = attn_slice(0, nq, fd)

            # S4 (hp1, nq0): last q chain + norm(S3).
            fd = merge(
                norm_fill(pending),
                chain_fill(
                    [(1, "k", 2), (1, "k", 3), (1, "q", 2), (1, "q", 3)],
                    (1, 4, 9, 12),
                ),
            )
            pending = attn_slice(1, 0, fd)

            # S5-S7 (hp1, nq1-3): previous slice's normalization, then that
            # query slice's 8 output-projection groups one per step.
            for nq in range(1, NQ):
                fd = norm_fill(pending)
                for mo in range(OUT // 128):
                    fd.setdefault(8 + mo, []).append(
                        lambda n=nq - 1, mo=mo: out_proj_group(n, mo)
                    )
                pending = attn_slice(1, nq, fd)

            # Tail: last normalization + last query slice's projection, with
            # no-dep dummy matmuls woven in so HAM never sees a >3.4us PE
            # idle gap (a cold tail ran at half clock in earlier traces).
            def warm(k=2):
                for _ in range(k):
                    wps = psp.tile([128, 1024], FP32, tag="S", name="warm")
                    nc.tensor.matmul(
                        wps[:, 0:512], lhsT=wu[:, 0:128], rhs=wu[:], start=True, stop=True
                    )

            # Tail normalization goes back through ACT ln/exp: here LATENCY
            # matters (nothing left to overlap) and the two ACT ops beat the
            # serial single-partition DVE Newton chain; the natural_log_exp
            # table set is already resident. ACT produces +1/d, so the
            # broadcast uses the +1 ones vector.
            p1, _seed_t, _newt_t, _p2a, _p2b = pending
            p1()
            warm(3)
            hp_t, nq_t = 1, NQ - 1
            cs_t = p1.st["cs"]
            ln_t = work.tile([1, 1024], FP32, tag="lnt", name="ln_tail")
            nc.scalar.activation(
                ln_t[:], cs_t[64:65, :], mybir.ActivationFunctionType.Ln
            )
            warm(3)
            rr_t = work.tile([1, 1024], FP16, tag="rrt", name="rr_tail")
            nc.scalar.activation(
                rr_t[:], ln_t[:], mybir.ActivationFunctionType.Exp, scale=-1.0
            )
            warm(3)
            for a in range(2):
                bc = pop.tile([128, 512], FP32, tag="po", name=f"bct{a}")
                nc.tensor.matmul(
                    bc[0:64, :],
                    lhsT=ones_p[:],
                    rhs=rr_t[0:1, 512 * a : 512 * a + 512],
                    start=True,
                    stop=True,
                )
                nc.vector.tensor_mul(
                    cns[hp_t][64 * a : 64 * a + 64, nq_t * 512 : (nq_t + 1) * 512],
                    cs_t[0:64, 512 * a : 512 * a + 512],
                    bc[0:64, :],
                )
            for mo in range(OUT // 128):
                out_proj_group(NQ - 1, mo, pool=(psp if mo % 2 else None))

    _split_waits(nc)
    return nc


_PROGRAM = None


def _get_program():
    global _PROGRAM
    if _PROGRAM is None:
        _PROGRAM = build_program()
    return _PROGRAM


def _shard_inputs(x, Wq, bq, Wk, bk, Wv, bv, Wo, bo):
    f16 = np.float16
    in_maps = []
    for c in range(NCORES):
        b = c // 4
        g = c % 4
        hs = slice(g * HPC, (g + 1) * HPC)

        xTc = np.ascontiguousarray(x[b].T).astype(f16)  # [D, S]
        wq_c = np.ascontiguousarray(Wq[hs].transpose(1, 0, 2).reshape(D, E)).astype(f16)
        wk_c = np.ascontiguousarray(Wk[hs].transpose(1, 0, 2).reshape(D, E)).astype(f16)
        wv_c = np.zeros((D, EV), dtype=np.float32)
        mv_c = np.zeros((1, EV), dtype=np.float32)
        for h in range(HPC):
            wv_c[:, h * 65 : h * 65 + 64] = Wv[hs][h]
            mv_c[0, h * 65 : h * 65 + 64] = bv[hs][h]
            mv_c[0, h * 65 + 64] = 1.0
        wo_c = np.ascontiguousarray(Wo[g * E : (g + 1) * E, :]).astype(f16)
        bq_c = np.ascontiguousarray(bq[hs].reshape(E // 128, 128).T).astype(np.float32)
        bk_c = np.ascontiguousarray(bk[hs].reshape(E // 128, 128).T).astype(np.float32)
        bo_c = np.ascontiguousarray(
            (bo.astype(np.float32) * 0.25).reshape(OUT // 128, 128).T
        ).astype(np.float32)

        in_maps.append(
            {
                "xT": xTc,
                "wq": wq_c,
                "wk": wk_c,
                "wv": wv_c.astype(f16),
                "wo": wo_c,
                "bq": bq_c,
                "bk": bk_c,
                "maskv": np.ascontiguousarray(
                    np.broadcast_to(mv_c, (128, EV))
                ).astype(np.float32),
                "bo4": bo_c,
            }
        )
    return in_maps


def kernel(x, Wq, bq, Wk, bk, Wv, bv, Wo, bo, _trace=False, _result_box=None):
    in_maps = _shard_inputs(
        np.asarray(x, np.float32),
        np.asarray(Wq, np.float32),
        np.asarray(bq, np.float32),
        np.asarray(Wk, np.float32),
        np.asarray(bk, np.float32),
        np.asarray(Wv, np.float32),
        np.asarray(bv, np.float32),
        np.asarray(Wo, np.float32),
        np.asarray(bo, np.float32),
    )
    nc = _get_program()
    res = run_bass_kernel_spmd(nc, in_maps, list(range(NCORES)), trace=_trace)
    if _result_box is not None:
        _result_box.append(res)

    out = np.empty((B, S, OUT), dtype=np.float32)
    for b in range(B):
        acc = res.results[4 * b]["outT"].astype(np.float32)
        for g in range(1, 4):
            acc = acc + res.results[4 * b + g]["outT"].astype(np.float32)
        out[b] = acc.T
    return out


# revision 54
# speedup vs baseline: 1.1976x; 1.0134x over previous
"""Multi-head attention (B=2, S=2048, D=1024, H=16, dh=64) on 8 Trainium2 cores.

Sharding: head-tensor-parallel x batch. Core c owns batch b=c//4 and heads
4*(c%4)..4*(c%4)+3 (256 of the 1024 ctx dims). Each core computes its heads'
Q/K/V projections, attention, and a partial output projection against its
256 rows of Wo (+ bo/4 so the 4 partials per batch sum to one bo). The host
unshard step sums the 4 partial outputs per batch (the tensor-parallel
all-reduce), done at gather time.

Per-core kernel (fp16 matmul operands, fp32 PSUM accumulation):
  qT/kT [256e, 2048t] = W.T @ x.T computed directly in transposed form so
  scores^T [kt, qt] = (kT slice).T @ (qT slice) needs no on-device transpose.
  Head pairs are row-packed (heads at array rows 0-63 / 64-127) so the two
  K=64 score matmuls of a pair run concurrently via tile_position row groups.
  exp runs on ScalarE with the 1/sqrt(dh) scale folded in. A@V uses a
  stationary operand [V | 1] (ones column injected by the eviction mask-add)
  so the softmax denominator falls out of the same matmul. The denominator
  reciprocal is 1/s = exp(-ln(s)) in two ScalarE ops (same table set as the
  scores exp), woven into the next slice behind queued score exps; the
  1/rowsum row is broadcast across partitions with a K=1 fp16 matmul.

Schedule: DMA order is arranged so the first exp fires ~7us in (wq, wk, x
column-slice 0 first; v/k/q projections stream in JIT as later x column
slices land). Projection/output-projection matmul chains are chopped into
2-3 matmul chunks woven between attention m-steps so the exp cadence never
breaks. Each attn1 slice carries the previous slice's normalization and one
query-slice of output projection; only the last q-slice's projection remains
for the tail.
"""

import numpy as np

import bass_rust
import concourse.bass as bass
import concourse.mybir as mybir
import concourse.tile as tile
from concourse.bass_utils import run_bass_kernel_spmd

B = 2
S = 2048
D = 1024
H = 16
DH = 64
OUT = 1024
NCORES = 8
HPC = H // 4  # heads per core = 4
E = HPC * DH  # 256 ctx dims per core
EV = HPC * (DH + 1)  # 260: v with interleaved ones columns

FP16 = mybir.dt.float16  # fp16: same PE rate as bf16, 10-bit mantissa
FP32 = mybir.dt.float32
F32R = mybir.dt.float32r
I32 = mybir.dt.int32
ALU = mybir.AluOpType

SCALE = 1.0 / float(np.sqrt(DH))
# Reciprocal bit-trick seed: bitcast(~i + RECIP_C) == -(~5%-accurate 1/d)
# for positive d (Schraudolph constant with the sign bit folded in); one
# Newton step squares the error to ~2.6e-3, well under the accuracy gate.
RECIP_C = int(np.int32(np.uint32((0x7EF311C3 + 0x80000000 + 1) & 0xFFFFFFFF)))

KT = D // 128  # 8 k-tiles for projections
MT = S // 128  # 16 key-token tiles
NQ = S // 512  # 4 query slices of 512


def _split_waits(nc, maxw=1):
    """This container's walrus rejects instructions carrying more than one
    semaphore wait ("Too many sync wait commands"); hoist extras onto
    standalone same-engine nops, preserving per-engine program order."""
    for bb in nc.main_func.blocks:
        new_il = []
        for inst in bb.instructions:
            si = inst.sync_info
            if si is not None and si.on_wait and len(si.on_wait) > maxw:
                waits = list(si.on_wait)
                for j, w in enumerate(waits[:-maxw]):
                    nop = mybir.InstNoOp(
                        name=f"{inst.name}-ws{j}", ins=[], outs=[], engine=inst.engine
                    )
                    nop.sync_info = bass_rust.SyncInfo(on_wait=[w], on_update=[])
                    new_il.append(nop)
                inst.sync_info = bass_rust.SyncInfo(
                    on_wait=waits[-maxw:], on_update=list(si.on_update)
                )
            new_il.append(inst)
        bb.instructions = new_il


def build_program():
    nc = bass.Bass()

    xT = nc.declare_dram_parameter("xT", [D, S], FP16, isOutput=False)
    wq = nc.declare_dram_parameter("wq", [D, E], FP16, isOutput=False)
    wk = nc.declare_dram_parameter("wk", [D, E], FP16, isOutput=False)
    wv = nc.declare_dram_parameter("wv", [D, EV], FP16, isOutput=False)
    wo = nc.declare_dram_parameter("wo", [E, OUT], FP16, isOutput=False)
    bqp = nc.declare_dram_parameter("bq", [128, E // 128], FP32, isOutput=False)
    bkp = nc.declare_dram_parameter("bk", [128, E // 128], FP32, isOutput=False)
    mvp = nc.declare_dram_parameter("maskv", [128, EV], FP32, isOutput=False)
    bop = nc.declare_dram_parameter("bo4", [128, OUT // 128], FP32, isOutput=False)
    outT = nc.declare_dram_parameter("outT", [OUT, S], FP16, isOutput=True)

    with tile.TileContext(nc) as tc:
        with (
            tc.tile_pool(name="w", bufs=1) as wpool,
            tc.tile_pool(name="work", bufs=3) as work,
            tc.tile_pool(name="cnp", bufs=1) as cnpool,
            tc.tile_pool(name="ps", bufs=2, space="PSUM") as psp,
            tc.tile_pool(name="ctxps", bufs=2, space="PSUM") as ctxp,
            tc.tile_pool(name="pop", bufs=2, space="PSUM") as pop,
        ):
            # ---- persistent SBUF residents ----
            xts = [wpool.tile([128, S], FP16, tag=f"xt{k}", name=f"xt{k}") for k in range(KT)]
            wqs = [wpool.tile([128, E], FP16, tag=f"wq{k}", name=f"wq{k}") for k in range(KT)]
            wks = [wpool.tile([128, E], FP16, tag=f"wk{k}", name=f"wk{k}") for k in range(KT)]
            wvs = [wpool.tile([128, EV], FP16, tag=f"wv{k}", name=f"wv{k}") for k in range(KT)]
            wos = [wpool.tile([128, OUT], FP16, tag=f"wo{k}", name=f"wo{k}") for k in range(2)]
            bq_s = wpool.tile([128, E // 128], FP32, tag="bq")
            bk_s = wpool.tile([128, E // 128], FP32, tag="bk")
            mv_s = wpool.tile([128, EV], FP32, tag="mv")
            bo_s = wpool.tile([128, OUT // 128], FP32, tag="bo")
            ones_f = wpool.tile([1, 64], FP16, tag="ones_f")
            ones_p = wpool.tile([1, 64], FP16, tag="ones_p")
            qts = [wpool.tile([128, S], FP16, tag=f"qt{m}", name=f"qt{m}") for m in range(2)]
            kts = [wpool.tile([128, S], FP16, tag=f"kt{m}", name=f"kt{m}") for m in range(2)]
            vts = [wpool.tile([128, EV], FP16, tag=f"vt{m}", name=f"vt{m}") for m in range(MT)]
            cns = [cnpool.tile([128, S], FP16, tag=f"cn{m}", name=f"cn{m}") for m in range(2)]

            # DMA issue costs ~650ns each and is serial per engine queue, so
            # the critical first transfers (wq, wk, x column-slice 0) are
            # spread across the three DMA-capable queues (Sync, GpSimd,
            # Scalar) to issue in parallel. Scalar gets only wk so the exp
            # table load / first exp aren't queued behind DMA issues.
            for k in range(KT):
                nc.gpsimd.dma_start(out=wqs[k][:], in_=wq[k * 128 : (k + 1) * 128, :])
            for k in range(KT):
                nc.scalar.dma_start(out=wks[k][:], in_=wk[k * 128 : (k + 1) * 128, :])
            for k in range(KT):
                nc.sync.dma_start(
                    out=xts[k][:, 0:512], in_=xT[k * 128 : (k + 1) * 128, 0:512]
                )
            nc.sync.dma_start(out=bq_s[:], in_=bqp[:])
            nc.sync.dma_start(out=bk_s[:], in_=bkp[:])
            for k in range(KT):
                nc.sync.dma_start(
                    out=xts[k][:, 512:1024], in_=xT[k * 128 : (k + 1) * 128, 512:1024]
                )
            for k in range(KT):
                nc.gpsimd.dma_start(out=wvs[k][:], in_=wv[k * 128 : (k + 1) * 128, :])
            nc.gpsimd.dma_start(out=mv_s[:], in_=mvp[:])
            for k in range(KT):
                nc.gpsimd.dma_start(
                    out=xts[k][:, 1024:1536], in_=xT[k * 128 : (k + 1) * 128, 1024:1536]
                )
            for k in range(KT):
                nc.sync.dma_start(
                    out=xts[k][:, 1536:2048], in_=xT[k * 128 : (k + 1) * 128, 1536:2048]
                )
            for k in range(2):
                nc.gpsimd.dma_start(out=wos[k][:], in_=wo[k * 128 : (k + 1) * 128, :])
            nc.gpsimd.dma_start(out=bo_s[:], in_=bop[:])
            nc.vector.memset(ones_f[:], -1.0)
            nc.vector.memset(ones_p[:], 1.0)

            # Warm the PE clock (HAM un-throttles after ~3.4us sustained)
            # while the first DMAs stream in: no-dependency dummy matmuls.
            wu = wpool.tile([128, 512], FP16, tag="wu")
            nc.vector.memset(wu[:], 0.0)
            wups = psp.tile([128, 1024], FP32, tag="S", name="wups")
            for i in range(16):
                nc.tensor.matmul(
                    wups[:, 0:512], lhsT=wu[:, 0:128], rhs=wu[:], start=True, stop=True
                )

            # ---- projection chain helpers ----
            def qk_burst(hp, which, n):
                """Full 8-matmul projection group (prologue / slice-0 use)."""
                w_s, dst, bias = (
                    (wqs, qts, bq_s) if which == "q" else (wks, kts, bk_s)
                )
                ps = pop.tile([128, 512], FP32, tag="po", name=f"qk_{which}{hp}{n}")
                for k in range(KT):
                    nc.tensor.matmul(
                        ps[:],
                        lhsT=w_s[k][:, hp * 128 : (hp + 1) * 128],
                        rhs=xts[k][:, n * 512 : (n + 1) * 512],
                        start=(k == 0),
                        stop=(k == KT - 1),
                    )
                nc.vector.tensor_add(
                    dst[hp][:, n * 512 : (n + 1) * 512],
                    ps[:],
                    bias[:, hp : hp + 1].to_broadcast((128, 512)),
                )

            def qk_chunks(hp, which, n):
                """The same group as 3 thunks (3+3+2 matmuls) to weave between
                attention m-steps. The accumulator PSUM tile persists across
                chunks (pop ring, bufs=1 serializes chains)."""
                w_s, dst, bias = (
                    (wqs, qts, bq_s) if which == "q" else (wks, kts, bk_s)
                )
                state = {}

                def chunk(ks, first, last):
                    def t():
                        if first:
                            state["ps"] = pop.tile(
                                [128, 512], FP32, tag="po", name=f"qkc_{which}{hp}{n}"
                            )
                        ps = state["ps"]
                        for k in ks:
                            nc.tensor.matmul(
                                ps[:],
                                lhsT=w_s[k][:, hp * 128 : (hp + 1) * 128],
                                rhs=xts[k][:, n * 512 : (n + 1) * 512],
                                start=(k == 0),
                                stop=(k == KT - 1),
                            )
                        if last:
                            nc.vector.tensor_add(
                                dst[hp][:, n * 512 : (n + 1) * 512],
                                ps[:],
                                bias[:, hp : hp + 1].to_broadcast((128, 512)),
                            )
                    return t

                return [
                    chunk(range(0, 3), True, False),
                    chunk(range(3, 6), False, False),
                    chunk(range(6, 8), False, True),
                ]

            def v_group(m):
                """v_ext rows m*128..m*128+127 (token-major); the ones columns
                (and bv) are injected by the mask-add eviction, so no bias
                matmul is needed."""
                ps = pop.tile([128, 512], FP32, tag="po", name=f"psv{m}")
                for k in range(KT):
                    nc.tensor.matmul(
                        ps[:, :EV],
                        lhsT=xts[k][:, m * 128 : (m + 1) * 128],
                        rhs=wvs[k][:],
                        start=(k == 0),
                        stop=(k == KT - 1),
                    )
                nc.vector.tensor_add(vts[m][:], ps[:, :EV], mv_s[:])

            def norm_thunks(ctx_a, ctx_b, hp, nq):
                """The deferred normalization of a finished slice as 5 thunks:
                [p1-evict (DVE), ln (ACT), 1/x exp (ACT), head-a broadcast+
                scale, head-b broadcast+scale]. Spreading them mid-next-slice
                keeps the ACT recip from stalling the score-exp cadence."""
                st = {}

                def p1():
                    st["cs"] = work.tile([65, 1024], FP32, tag="cs", bufs=3, name="cs_ab")
                    nc.vector.tensor_copy(st["cs"][:, 0:512], ctx_a[:])
                    nc.vector.tensor_copy(st["cs"][:, 512:1024], ctx_b[:])
                    # Denominator row replicated to a partition-0 tile (PSUM
                    # source, so the partition remap is legal): the DVE
                    # Newton ops need SBUF operands at matching bases.
                    st["dd"] = work.tile([1, 1024], FP32, tag="dd", bufs=3, name="dd_ab")
                    nc.vector.tensor_copy(st["dd"][0:1, 0:512], ctx_a[64:65, :])
                    nc.vector.tensor_copy(st["dd"][0:1, 512:1024], ctx_b[64:65, :])

                def seed_t():
                    # z = bitcast(~i + C) ~= -1/d, entirely on DVE: the
                    # pacing ACT engine never runs the reciprocal.
                    st["z"] = work.tile([1, 1024], FP32, tag="zz", bufs=3, name="z_ab")
                    zn = work.tile([1, 1024], FP32, tag="zn", bufs=3, name="zn_ab")
                    nc.vector.tensor_scalar(
                        zn[:].bitcast(I32), st["dd"][:].bitcast(I32),
                        0, None, ALU.bitwise_not,
                    )
                    nc.vector.tensor_scalar(
                        st["z"][:].bitcast(I32), zn[:].bitcast(I32),
                        RECIP_C, None, ALU.add,
                    )

                def newt_t():
                    # One Newton step straight to fp16: rr = (d*z + 2)*z,
                    # still -1/d; the -1 ones vector in the broadcast matmul
                    # flips the sign back.
                    tt = work.tile([1, 1024], FP32, tag="tt", bufs=3, name="t_ab")
                    st["rr"] = work.tile([1, 1024], FP16, tag="rr", bufs=3, name="rr_ab")
                    nc.vector.tensor_mul(tt[:], st["dd"][:], st["z"][:])
                    nc.vector.scalar_tensor_tensor(
                        st["rr"][:], tt[:], 2.0, st["z"][:],
                        op0=ALU.add, op1=ALU.mult,
                    )

                def p2(a):
                    def t():
                        bc = pop.tile([128, 512], FP32, tag="po", name=f"bc{hp}{a}{nq}")
                        nc.tensor.matmul(
                            bc[0:64, :],
                            lhsT=ones_f[:],
                            rhs=st["rr"][0:1, 512 * a : 512 * a + 512],
                            start=True,
                            stop=True,
                        )
                        nc.vector.tensor_mul(
                            cns[hp][64 * a : 64 * a + 64, nq * 512 : (nq + 1) * 512],
                            st["cs"][0:64, 512 * a : 512 * a + 512],
                            bc[0:64, :],
                        )
                    return t

                p1.st = st  # the tail reads the cs tile out of the closure
                return [p1, seed_t, newt_t, p2(0), p2(1)]

            def out_proj_group(n, mo, pool=None):
                """One [128,512] tile of the partial out^T for query slice n.
                Output DMA issues alternate between the Sync and GpSimd
                queues (both idle here) so the tail's last issues aren't
                serialized on one queue. `pool` lets the tail alternate PSUM
                rings so back-to-back groups pipeline instead of serializing
                on the single pop buffer."""
                ps = (pool or pop).tile(
                    [128, 512], FP32, tag="S" if pool is not None else "po",
                    name=f"op{n}{mo}",
                )
                for k in range(2):
                    nc.tensor.matmul(
                        ps[:],
                        lhsT=wos[k][:, mo * 128 : (mo + 1) * 128],
                        rhs=cns[k][:, n * 512 : (n + 1) * 512],
                        start=(k == 0),
                        stop=(k == 1),
                    )
                ot = work.tile([128, 512], FP16, tag="ot")
                nc.vector.tensor_scalar_add(ot[:], ps[:], bo_s[:, mo : mo + 1])
                eng = nc.sync if mo % 2 == 0 else nc.gpsimd
                eng.dma_start(
                    out=outT[mo * 128 : (mo + 1) * 128, n * 512 : (n + 1) * 512],
                    in_=ot[:],
                )

            def attn_slice(hp, nq, fillers):
                """One query slice (512) of attention for head pair hp.
                fillers: dict m -> list of thunks emitted just before step m.
                Returns (cs, rr, hp, nq) for the deferred normalization."""
                ctx_a = ctxp.tile([65, 512], FP32, tag="ctx", name="ctx_a")
                ctx_b = ctxp.tile([65, 512], FP32, tag="ctx", name="ctx_b")
                for m in range(MT):
                    sps = psp.tile([128, 1024], FP32, tag="S", name="sps")
                    nc.tensor.matmul(
                        sps[:, 0:512],
                        lhsT=kts[hp][0:64, m * 128 : (m + 1) * 128],
                        rhs=qts[hp][0:64, nq * 512 : (nq + 1) * 512],
                        start=True,
                        stop=True,
                    )
                    nc.tensor.matmul(
                        sps[:, 512:1024],
                        lhsT=kts[hp][64:128, m * 128 : (m + 1) * 128],
                        rhs=qts[hp][64:128, nq * 512 : (nq + 1) * 512],
                        start=True,
                        stop=True,
                    )
                    ee = work.tile([128, 1024], FP16, tag="E", bufs=4)
                    nc.scalar.activation(
                        ee[:], sps[:], mybir.ActivationFunctionType.Exp, scale=SCALE
                    )
                    # Fillers go BETWEEN the scores pair and the A@V pair on
                    # the PE queue: the next step's scores (and so the exp
                    # cadence) never queue behind filler work; only the AVs
                    # slip, absorbed by the ee ring.
                    for f in fillers.get(m, ()):
                        f()
                    ha = 2 * hp
                    nc.tensor.matmul(
                        ctx_a[:],
                        lhsT=vts[m][:, ha * 65 : ha * 65 + 65],
                        rhs=ee[:, 0:512],
                        start=(m == 0),
                        stop=(m == MT - 1),
                    )
                    nc.tensor.matmul(
                        ctx_b[:],
                        lhsT=vts[m][:, (ha + 1) * 65 : (ha + 1) * 65 + 65],
                        rhs=ee[:, 512:1024],
                        start=(m == 0),
                        stop=(m == MT - 1),
                    )
                return norm_thunks(ctx_a, ctx_b, hp, nq)

            def merge(*fds):
                out = {}
                for fd in fds:
                    for k, v in fd.items():
                        out[k] = out.get(k, []) + list(v)
                return out

            def norm_fill(pending, at=(0, 1, 2, 6, 7)):
                """Weave the previous slice's normalization pipeline into
                this slice: DVE evict at 0, ACT ln/exp at 4/5 (behind a few
                queued score exps so the ACT never starves the cadence),
                broadcasts at 6/7."""
                return {s: [t] for s, t in zip(at, pending)}

            def chain_fill(chains, starts):
                """Place each chain's 3 chunks at steps s, s+1, s+2."""
                fd = {}
                for (hp, which, n), s in zip(chains, starts):
                    for i, t in enumerate(qk_chunks(hp, which, n)):
                        fd.setdefault(s + i, []).append(t)
                return fd

            # ---- emission schedule ----
            # Prologue: q0 n=0 and k0 n=0, k-interleaved so both chains ride
            # the same DMA wave (each x k-tile feeds both the q and k mm).
            ps_q = pop.tile([128, 512], FP32, tag="po", name="qk_q00")
            ps_k = pop.tile([128, 512], FP32, tag="po", name="qk_k00")
            for k in range(KT):
                nc.tensor.matmul(
                    ps_q[:], lhsT=wqs[k][:, 0:128], rhs=xts[k][:, 0:512],
                    start=(k == 0), stop=(k == KT - 1),
                )
                nc.tensor.matmul(
                    ps_k[:], lhsT=wks[k][:, 0:128], rhs=xts[k][:, 0:512],
                    start=(k == 0), stop=(k == KT - 1),
                )
            nc.vector.tensor_add(
                qts[0][:, 0:512], ps_q[:], bq_s[:, 0:1].to_broadcast((128, 512))
            )
            nc.vector.tensor_add(
                kts[0][:, 0:512], ps_k[:], bk_s[:, 0:1].to_broadcast((128, 512))
            )

            # S0 (hp0, nq0): v chains JIT per step (DMA-paced); k0 n=1..3 and
            # q0 n=1 burst in when their x column slices land.
            fill0 = {m: [lambda m=m: v_group(m)] for m in range(MT)}
            fill0[3] = [lambda: qk_burst(0, "k", 1)] + fill0[3]
            fill0[6] = [lambda: qk_burst(0, "k", 2)] + fill0[6]
            fill0[9] = [lambda: qk_burst(0, "k", 3)] + fill0[9]
            fill0[11] = [lambda: qk_burst(0, "q", 1)] + fill0[11]
            pending = attn_slice(0, 0, fill0)

            # S1-S3 (hp0, nq1-3): remaining projections as woven 3-chunk
            # chains + previous slice's normalization pipeline.
            # Two chains per slice here; the rest ride in S4, which carries
            # no output-projection work and has the most PE slack.
            plans = {
                1: [(0, "q", 2), (1, "k", 0)],
                2: [(0, "q", 3), (1, "k", 1)],
                3: [(1, "q", 0), (1, "q", 1)],
            }
            for nq in range(1, NQ):
                fd = merge(
                    norm_fill(pending),
                    chain_fill(plans[nq], (2, 9)),
                )
                pending = attn_slice(0, nq, fd)

            # S4 (hp1, nq0): last q chain + norm(S3).
            fd = merge(
                norm_fill(pending),
                chain_fill(
                    [(1, "k", 2), (1, "k", 3), (1, "q", 2), (1, "q", 3)],
                    (1, 4, 9, 12),
                ),
            )
            pending = attn_slice(1, 0, fd)

            # S5-S7 (hp1, nq1-3): previous slice's normalization, then that
            # query slice's 8 output-projection groups one per step.
            for nq in range(1, NQ):
                fd = norm_fill(pending)
                for mo in range(OUT // 128):
                    fd.setdefault(8 + mo, []).append(
                        lambda n=nq - 1, mo=mo: out_proj_group(n, mo)
                    )
                pending = attn_slice(1, nq, fd)

            # Tail: last normalization + last query slice's projection, with
            # no-dep dummy matmuls woven in so HAM never sees a >3.4us PE
            # idle gap (a cold tail ran at half clock in earlier traces).
            def warm(k=2):
                for _ in range(k):
                    wps = psp.tile([128, 1024], FP32, tag="S", name="warm")
                    nc.tensor.matmul(
                        wps[:, 0:512], lhsT=wu[:, 0:128], rhs=wu[:], start=True, stop=True
                    )

            # Tail normalization goes back through ACT ln/exp: here LATENCY
            # matters (nothing left to overlap) and the two ACT ops beat the
            # serial single-partition DVE Newton chain; the natural_log_exp
            # table set is already resident. ACT produces +1/d, so the
            # broadcast uses the +1 ones vector.
            p1, _seed_t, _newt_t, _p2a, _p2b = pending
            p1()
            warm(3)
            hp_t, nq_t = 1, NQ - 1
            cs_t = p1.st["cs"]
            ln_t = work.tile([1, 1024], FP32, tag="lnt", name="ln_tail")
            nc.scalar.activation(
                ln_t[:], cs_t[64:65, :], mybir.ActivationFunctionType.Ln
            )
            warm(3)
            rr_t = work.tile([1, 1024], FP16, tag="rrt", name="rr_tail")
            nc.scalar.activation(
                rr_t[:], ln_t[:], mybir.ActivationFunctionType.Exp, scale=-1.0
            )
            warm(3)
            for a in range(2):
                bc = pop.tile([128, 512], FP32, tag="po", name=f"bct{a}")
                nc.tensor.matmul(
                    bc[0:64, :],
                    lhsT=ones_p[:],
                    rhs=rr_t[0:1, 512 * a : 512 * a + 512],
                    start=True,
                    stop=True,
                )
                nc.vector.tensor_mul(
                    cns[hp_t][64 * a : 64 * a + 64, nq_t * 512 : (nq_t + 1) * 512],
                    cs_t[0:64, 512 * a : 512 * a + 512],
                    bc[0:64, :],
                )
            for mo in range(OUT // 128):
                out_proj_group(NQ - 1, mo, pool=(psp if mo % 2 else None))

    _split_waits(nc)
    return nc


_PROGRAM = None


def _get_program():
    global _PROGRAM
    if _PROGRAM is None:
        _PROGRAM = build_program()
    return _PROGRAM


def _shard_inputs(x, Wq, bq, Wk, bk, Wv, bv, Wo, bo):
    f16 = np.float16
    in_maps = []
    for c in range(NCORES):
        b = c // 4
        g = c % 4
        hs = slice(g * HPC, (g + 1) * HPC)

        xTc = np.ascontiguousarray(x[b].T).astype(f16)  # [D, S]
        wq_c = np.ascontiguousarray(Wq[hs].transpose(1, 0, 2).reshape(D, E)).astype(f16)
        wk_c = np.ascontiguousarray(Wk[hs].transpose(1, 0, 2).reshape(D, E)).astype(f16)
        wv_c = np.zeros((D, EV), dtype=np.float32)
        mv_c = np.zeros((1, EV), dtype=np.float32)
        for h in range(HPC):
            wv_c[:, h * 65 : h * 65 + 64] = Wv[hs][h]
            mv_c[0, h * 65 : h * 65 + 64] = bv[hs][h]
            mv_c[0, h * 65 + 64] = 1.0
        wo_c = np.ascontiguousarray(Wo[g * E : (g + 1) * E, :]).astype(f16)
        bq_c = np.ascontiguousarray(bq[hs].reshape(E // 128, 128).T).astype(np.float32)
        bk_c = np.ascontiguousarray(bk[hs].reshape(E // 128, 128).T).astype(np.float32)
        bo_c = np.ascontiguousarray(
            (bo.astype(np.float32) * 0.25).reshape(OUT // 128, 128).T
        ).astype(np.float32)

        in_maps.append(
            {
                "xT": xTc,
                "wq": wq_c,
                "wk": wk_c,
                "wv": wv_c.astype(f16),
                "wo": wo_c,
                "bq": bq_c,
                "bk": bk_c,
                "maskv": np.ascontiguousarray(
                    np.broadcast_to(mv_c, (128, EV))
                ).astype(np.float32),
                "bo4": bo_c,
            }
        )
    return in_maps


def kernel(x, Wq, bq, Wk, bk, Wv, bv, Wo, bo, _trace=False, _result_box=None):
    in_maps = _shard_inputs(
        np.asarray(x, np.float32),
        np.asarray(Wq, np.float32),
        np.asarray(bq, np.float32),
        np.asarray(Wk, np.float32),
        np.asarray(bk, np.float32),
        np.asarray(Wv, np.float32),
        np.asarray(bv, np.float32),
        np.asarray(Wo, np.float32),
        np.asarray(bo, np.float32),
    )
    nc = _get_program()
    res = run_bass_kernel_spmd(nc, in_maps, list(range(NCORES)), trace=_trace)
    if _result_box is not None:
        _result_box.append(res)

    out = np.empty((B, S, OUT), dtype=np.float32)
    for b in range(B):
        acc = res.results[4 * b]["outT"].astype(np.float32)
        for g in range(1, 4):
            acc = acc + res.results[4 * b + g]["outT"].astype(np.float32)
        out[b] = acc.T
    return out
